# revision 1
# baseline (speedup 1.0000x reference)
"""Bass/Trainium2 kernel for the attention-decoder problem (v2).

Data-parallel over batch: 8 cores x 32 batch each. Per core, a 26-step
Bahdanau-attention + GRU scan over T=128 encoder positions, then a
generator GEMM [832, 512] @ [512, 6736].

v2 structure vs the v1 baseline:
- e is computed TRANSPOSED ([t | b] in PSUM) via per-b matmuls with the
  tanh'd z tile as stationary and w_score as a 1-column moving operand.
  This kills the transpose-DMA, the big exp (free size 32 instead of
  4096), and all DVE softmax reductions.
- softmax denominator s = sum_t exp(e) comes from a ones-stationary
  matmul off exp(e^T); 1/s is folded into the PSUM->SBUF copy of the
  context (context is computed from unnormalized exp(e)).
- the generator GEMM is interleaved into the scan: rows for steps
  [4g, 4g+4) are emitted during steps 4g+4..4g+7, filling the PE while
  the scalar engine runs tanh. Output rows stream out via DMA per
  512-class chunk.
"""

import numpy as np
import ml_dtypes

import concourse.bass as bass
import concourse.mybir as mybir
import concourse.tile as tile
from concourse import bacc
from concourse.bass_utils import run_bass_kernel_spmd

BF16 = mybir.dt.bfloat16
F32 = mybir.dt.float32
AF = mybir.ActivationFunctionType
ALU = mybir.AluOpType

T, BFULL, C = 128, 256, 512
H, L, NCLS = 512, 26, 6736
NCORES = 8
B = BFULL // NCORES          # 32 per core
HC = H // 128                # 4 h-chunks
CC = C // 128                # 4 c-chunks
G3 = 3 * H                   # 1536
NSUB = 512                   # class-chunk width (one PSUM bank of f32)
NSUBS = [(i * NSUB, min(NSUB, NCLS - i * NSUB))
         for i in range((NCLS + NSUB - 1) // NSUB)]     # 14 chunks


def _gen_schedule():
    """sched[k] = list of (k0, nk, off, width) generator sub-GEMMs emitted
    during scan step k. Group g covers steps [4g, 4g+4) (g=0..5); its 14
    class-chunks are spread over steps 4g+4..4g+7 (groups 0..4) or steps
    24..25 (group 5). Group 6 (steps 24..25) runs after the scan."""
    sched = {k: [] for k in range(L)}
    for g in range(6):
        k0, nk = 4 * g, 4
        steps = [4 * g + 4 + i for i in range(4)] if g < 5 else [24, 25]
        for i, (off, width) in enumerate(NSUBS):
            sched[steps[i % len(steps)]].append((k0, nk, off, width))
    return sched

GEN_SCHED = _gen_schedule()
GEN_TAIL = [(24, 2, off, width) for (off, width) in NSUBS]


def build_graph(reps=1, skip=()):
    nc = bacc.Bacc(None, target_bir_lowering=False, debug=False)

    # ---- DRAM parameters (per-core shard shapes) ----
    featsT = nc.declare_dram_parameter("featsT", [C, T, B], BF16, isOutput=False)
    featsS = nc.declare_dram_parameter("featsS", [B, T, C], BF16, isOutput=False)
    wi2h = nc.declare_dram_parameter("wi2h", [C, H], BF16, isOutput=False)
    wh2h = nc.declare_dram_parameter("wh2h", [H, H], BF16, isOutput=False)
    bh2h = nc.declare_dram_parameter("bh2h", [128, HC], F32, isOutput=False)
    wcol = nc.declare_dram_parameter("wcol", [128, HC], BF16, isOutput=False)
    wih = nc.declare_dram_parameter("wih", [C, G3], BF16, isOutput=False)
    whh = nc.declare_dram_parameter("whh", [H, G3], BF16, isOutput=False)
    brow = nc.declare_dram_parameter("brow", [1, 4 * H], BF16, isOutput=False)
    wgen = nc.declare_dram_parameter("wgen", [H, NCLS], BF16, isOutput=False)
    bgen = nc.declare_dram_parameter("bgen", [1, NCLS], BF16, isOutput=False)
    ident = nc.declare_dram_parameter("ident", [128, 128], BF16, isOutput=False)
    out = nc.declare_dram_parameter("out", [L, B, NCLS], F32, isOutput=True)

    with tile.TileContext(nc) as tc:
      for rep in range(reps):
        with (
            tc.tile_pool(name=f"pp{rep}", bufs=1) as pp,
            tc.tile_pool(name=f"zp{rep}", bufs=4) as zp,
            tc.tile_pool(name=f"gop{rep}", bufs=2) as gop,
            tc.tile_pool(name=f"psSm{rep}", bufs=2, space="PSUM") as psSm,
            tc.tile_pool(name=f"psC{rep}", bufs=2, space="PSUM") as psC,
            tc.tile_pool(name=f"psG{rep}", bufs=2, space="PSUM") as psG,
            tc.tile_pool(name=f"psO{rep}", bufs=2, space="PSUM") as psO,
        ):
            # ---- persistent SBUF ----
            P_sb = pp.tile([128, HC, T, B], BF16)      # i2h proj, [h | t, b]
            fs_sb = pp.tile([128, B, C], BF16)         # feats [t | b, c]
            wh2h_sb = pp.tile([128, HC, H], BF16)
            bh2h_sb = pp.tile([128, HC], F32)
            wcol_sb = pp.tile([128, HC], BF16)
            wih_sb = pp.tile([128, CC, G3], BF16)
            whh_sb = pp.tile([128, HC, G3], BF16)
            brow_sb = pp.tile([1, 4 * H], BF16)
            wgen_sb = pp.tile([128, HC, NCLS], BF16)
            # h^T ring buffer, slot-major: slot k%8 holds h_k^T. Generator
            # groups start at k0 in {0,4,...}, so group slots [k0%8, k0%8+nk)
            # are contiguous and never collide with the 4 writing steps.
            hist = pp.tile([128, HC, 8, B], BF16)
            h_bh = pp.tile([B, H], BF16)
            hpT_sb = pp.tile([128, HC, B], BF16)
            et_sb = pp.tile([128, B], F32)             # e^T accumulator
            # masked exp(e) tile: flat windows [32b, 32b+32) are one-hot
            # stationaries with exp(e[t, b]) at column b, zeros elsewhere
            M3_sb = pp.tile([128, B, 33], BF16)
            rcol_sb = pp.tile([B, 1], F32)             # 1/s per batch row
            # doubles as the gate n-preactivation buffer (disjoint liveness)
            cAB_sb = pp.tile([B, C], F32)
            ctx_bh = pp.tile([B, C], BF16)             # context [b | c]
            ctxT_sb = pp.tile([128, CC, B], BF16)
            rz_sb = pp.tile([B, 2 * H], BF16)
            Bn_sb = pp.tile([B, H], F32)
            ones128_sb = pp.tile([128, 1], BF16)
            ones1b_sb = pp.tile([1, 128], BF16)
            ident_sb = pp.tile([B, B], BF16)

            # ---- constant / weight loads ----
            nc.vector.memset(ones128_sb[:, :], 1.0)
            nc.vector.memset(ones1b_sb[:, :], 1.0)
            nc.vector.memset(M3_sb[:, :, :], 0.0)
            nc.vector.memset(hist[:, :, 7, :], 0.0)   # h_{-1} = 0 at slot 7
            nc.vector.memset(h_bh[:, :], 0.0)
            for kc in range(HC):
                nc.sync.dma_start(wh2h_sb[:, kc, :], wh2h[kc * 128:(kc + 1) * 128, :])
            nc.sync.dma_start(bh2h_sb[:, :], bh2h[:, :])
            nc.sync.dma_start(wcol_sb[:, :], wcol[:, :])
            nc.sync.dma_start(ident_sb[:, :], ident[0:B, 0:B])
            for b in range(B):
                nc.sync.dma_start(fs_sb[:, b, :], featsS[b, :, :])
            for kc in range(CC):
                nc.sync.dma_start(wih_sb[:, kc, :], wih[kc * 128:(kc + 1) * 128, :])
            for kc in range(HC):
                nc.sync.dma_start(whh_sb[:, kc, :], whh[kc * 128:(kc + 1) * 128, :])
            nc.sync.dma_start(brow_sb[:, :], brow[:, :])
            for kc in range(HC):
                nc.sync.dma_start(wgen_sb[:, kc, :], wgen[kc * 128:(kc + 1) * 128, :])

            # ---- prologue: P^T = W_i2h^T @ feats^T, laid out [h | t, b] ----
            with tc.tile_pool(name=f"ftp{rep}", bufs=2) as fp:
                wi2h_sb = fp.tile([128, CC, H], BF16, tag="w", bufs=1)
                for kc in range(CC):
                    nc.sync.dma_start(wi2h_sb[:, kc, :],
                                      wi2h[kc * 128:(kc + 1) * 128, :])
                for ns in range(16):       # 8 t x 32 b = 256 cols per sub
                    ft_ns = fp.tile([128, CC, 8, B], BF16, tag="ft", bufs=1)
                    for kc in range(CC):
                        nc.sync.dma_start(
                            ft_ns[:, kc, :, :],
                            featsT[kc * 128:(kc + 1) * 128,
                                   ns * 8:(ns + 1) * 8, :])
                    for mp in range(0, HC, 2):
                        pps = [psO.tile([128, 256], F32, tag="gen",
                                        name=f"ppsum{q}") for q in (mp, mp + 1)]
                        for kc in range(CC):
                            for mc, ppsum in zip((mp, mp + 1), pps):
                                nc.tensor.matmul(
                                    ppsum[:, :],
                                    wi2h_sb[:, kc, mc * 128:(mc + 1) * 128],
                                    ft_ns[:, kc, :, :],
                                    start=(kc == 0), stop=(kc == CC - 1))
                        for mc, ppsum in zip((mp, mp + 1), pps):
                            dst = P_sb[:, mc, ns * 8:(ns + 1) * 8, :]
                            if mc % 2 == 0:
                                nc.vector.tensor_copy(dst, ppsum[:, :])
                            else:
                                nc.scalar.activation(dst, ppsum[:, :], AF.Copy)

            # ---- scan ----
            for k in range(L):
                def hprev(kc, _k=k):
                    return hist[:, kc, (_k - 1) % 8, :]

                # hp^T = W_h2h^T h^T + b_h2h  -> [h' | b]
                for mp in range(0, HC, 2):
                    hps = [psSm.tile([128, B], F32, tag="sm", name=f"hp_ps{q}")
                           for q in (mp, mp + 1)]
                    for kc in range(HC):
                        for mc, hp_ps in zip((mp, mp + 1), hps):
                            nc.tensor.matmul(
                                hp_ps[:, :],
                                wh2h_sb[:, kc, mc * 128:(mc + 1) * 128],
                                hprev(kc),
                                start=(kc == 0), stop=(kc == HC - 1))
                    for mc, hp_ps in zip((mp, mp + 1), hps):
                        nc.scalar.activation(hpT_sb[:, mc, :], hp_ps[:, :],
                                             AF.Identity,
                                             bias=bh2h_sb[:, mc:mc + 1])

                # z = tanh(P + hp) per h-chunk
                def make_z(hc):
                    z = zp.tile([128, T, B], BF16, tag="z", name=f"z{hc}")
                    nc.vector.tensor_tensor(
                        z[:, :, :], P_sb[:, hc, :, :],
                        hpT_sb[:, hc:hc + 1, :].broadcast_to((128, T, B)),
                        op=ALU.add)
                    if "tanh" not in skip:
                        nc.scalar.activation(z[:, :, :], z[:, :, :], AF.Tanh)
                    return z

                z_tiles = [make_z(0), make_z(1)]

                # generator sub-GEMMs for a finished 4-step group (PE runs
                # these while Act does tanh; DVE bias-adds follow z-adds)
                gen_work = []
                gen_items = [] if "gen" in skip else GEN_SCHED[k]
                bges = []
                for (k0, nk, off, width) in gen_items:
                    bge = gop.tile([1, NSUB], BF16, tag="bg", bufs=4)
                    nc.sync.dma_start(bge[:, 0:width], bgen[:, off:off + width])
                    bges.append(bge)
                for pi in range(0, len(gen_items), 2):
                    pair = list(range(pi, min(pi + 2, len(gen_items))))
                    opss = [psO.tile([128, NSUB], F32, tag="gen", name=f"o_ps{q}")
                            for q in pair]
                    for hc in range(HC):
                        for q, o_ps in zip(pair, opss):
                            (k0, nk, off, width) = gen_items[q]
                            m = B * nk
                            s0 = k0 % 8
                            nc.tensor.matmul(
                                o_ps[:m, 0:width],
                                hist[:, hc, s0:s0 + nk, :],
                                wgen_sb[:, hc, off:off + width],
                                start=(hc == 0), stop=False)
                    for q, o_ps in zip(pair, opss):
                        (k0, nk, off, width) = gen_items[q]
                        m = B * nk
                        nc.tensor.matmul(
                            o_ps[:m, 0:width], ones1b_sb[0:1, 0:m],
                            bges[q][0:1, 0:width], start=False, stop=True)
                        gen_work.append((o_ps, k0, nk, off, width))

                # e^T[t, b] = sum_h w[h] z[h, t, b]: per-b matmuls with z as
                # stationary, accumulated in two contiguous-group halves
                # (interleaved PSUM groups mis-accumulate on HW) so the first
                # half overlaps the tanh of chunks 2-3.
                if "e" in skip:
                    nc.vector.memset(et_sb[:, :], 0.5)
                e01_ps = None if "e" in skip else psSm.tile([128, B], F32, tag="sm")
                for b in ([] if "e" in skip else range(B)):
                    for hc in range(2):
                        nc.tensor.matmul(
                            e01_ps[:, b:b + 1], z_tiles[hc][:, :, b],
                            wcol_sb[:, hc:hc + 1],
                            start=(hc == 0), stop=(hc == 1))
                z_tiles += [make_z(2), make_z(3)]
                if "e" not in skip:
                    e23_ps = psSm.tile([128, B], F32, tag="sm")
                    for b in range(B):
                        for hc in range(2, HC):
                            nc.tensor.matmul(
                                e23_ps[:, b:b + 1], z_tiles[hc][:, :, b],
                                wcol_sb[:, hc:hc + 1],
                                start=(hc == 2), stop=(hc == HC - 1))
                    nc.vector.tensor_copy(et_sb[:, :], e01_ps[:, :])
                    nc.vector.tensor_tensor(et_sb[:, :], et_sb[:, :], e23_ps[:, :],
                                            op=ALU.add)

                # exp(e) lands on the stride-33 diagonal of M3, making each
                # flat window [32b, 32b+32) a one-hot-masked stationary.
                nc.scalar.activation(M3_sb[:, :, 0], et_sb[:, :], AF.Exp)
                s_ps = psSm.tile([B, 1], F32, tag="sm")
                nc.tensor.matmul(s_ps[:, :], M3_sb[:, :, 0], ones128_sb[:, :],
                                 start=True, stop=True)
                nc.vector.reciprocal(rcol_sb[:, :], s_ps[:, :])

                # context[b, c] = sum_b' (masked-E_b' stationary) @ feats_b':
                # each matmul accumulates only row b; 32 stream-bound n=512
                # matmuls replace 128 ldweights-bound n=1 pairs.
                M3f = M3_sb[:, :, :].rearrange("p b c -> p (b c)")
                ctxA = psC.tile([B, C], F32, tag="ctx")
                ctxB = psC.tile([B, C], F32, tag="ctx")
                for j in ([] if "ctx" in skip else range(B // 2)):
                    for (p, b) in ((ctxA, 2 * j), (ctxB, 2 * j + 1)):
                        nc.tensor.matmul(
                            p[:, :], M3f[:, 32 * b:32 * b + 32],
                            fs_sb[:, b, :],
                            start=(j == 0), stop=(j == B // 2 - 1))
                if "ctx" in skip:
                    nc.vector.memset(ctxT_sb[:, :, :], 0.01)
                else:
                    nc.vector.tensor_copy(cAB_sb[:, :], ctxA[:, :])
                    nc.vector.tensor_tensor(cAB_sb[:, :], cAB_sb[:, :],
                                            ctxB[:, :], op=ALU.add)
                    nc.scalar.activation(ctx_bh[:, :], cAB_sb[:, :], AF.Copy,
                                         scale=rcol_sb[:, 0:1])
                    for cc in range(CC):
                        tc_ps = psSm.tile([128, B], BF16, tag="sm")
                        nc.tensor.transpose(
                            tc_ps[:, :], ctx_bh[:, cc * 128:(cc + 1) * 128],
                            ident_sb[:, :])
                        nc.vector.tensor_copy(ctxT_sb[:, cc, :], tc_ps[:, :])

                # generator bias-adds + output DMA (after z-adds on DVE)
                for gi, (o_ps, k0, nk, off, width) in enumerate(gen_work):
                    m = B * nk
                    o_sb = gop.tile([128, NSUB], F32, tag="ob")
                    if gi % 2 == 0:
                        nc.vector.tensor_copy(o_sb[:m, 0:width], o_ps[:m, 0:width])
                    else:
                        nc.scalar.activation(o_sb[:m, 0:width], o_ps[:m, 0:width],
                                             AF.Copy)
                    nc.sync.dma_start(
                        out[k0:k0 + nk, :, off:off + width], o_sb[:m, 0:width])

                # GRU gates. Biases ride into PSUM as k=1 matmuls; sigmoid is
                # computed as 0.5*tanh(x/2)+0.5 so the whole scan stays on the
                # exp_and_others activation table (tanh+exp) — no table loads.
                if "gates" not in skip:
                    gA = psG.tile([B, NSUB], F32, tag="g")
                    gB = psG.tile([B, NSUB], F32, tag="g")
                    for kc in range(CC):
                        for half, g_ps in ((0, gA), (1, gB)):
                            nc.tensor.matmul(
                                g_ps[:, :], ctxT_sb[:, kc, :],
                                wih_sb[:, kc, half * NSUB:(half + 1) * NSUB],
                                start=(kc == 0), stop=False)
                    for kc in range(HC):
                        for half, g_ps in ((0, gA), (1, gB)):
                            nc.tensor.matmul(
                                g_ps[:, :], hprev(kc),
                                whh_sb[:, kc, half * NSUB:(half + 1) * NSUB],
                                start=False, stop=False)
                    for half, g_ps in ((0, gA), (1, gB)):
                        nc.tensor.matmul(
                            g_ps[:, :], ones1b_sb[0:1, 0:B],
                            brow_sb[0:1, half * NSUB:(half + 1) * NSUB],
                            start=False, stop=True)
                        nc.scalar.activation(
                            rz_sb[:, half * NSUB:(half + 1) * NSUB], g_ps[:, :],
                            AF.Tanh, scale=0.5)
                nc.vector.tensor_scalar(rz_sb[:, :], rz_sb[:, :], 0.5, 0.5,
                                        ALU.mult, ALU.add)

                if "gates" in skip:
                    continue
                gn_ps = psG.tile([B, NSUB], F32, tag="g")
                hn_ps = psG.tile([B, NSUB], F32, tag="g")
                for kc in range(CC):
                    nc.tensor.matmul(gn_ps[:, :], ctxT_sb[:, kc, :],
                                     wih_sb[:, kc, 2 * H:], start=(kc == 0),
                                     stop=False)
                    nc.tensor.matmul(hn_ps[:, :], hprev(kc),
                                     whh_sb[:, kc, 2 * H:], start=(kc == 0),
                                     stop=False)
                nc.tensor.matmul(gn_ps[:, :], ones1b_sb[0:1, 0:B],
                                 brow_sb[0:1, 2 * H:3 * H],
                                 start=False, stop=True)
                nc.tensor.matmul(hn_ps[:, :], ones1b_sb[0:1, 0:B],
                                 brow_sb[0:1, 3 * H:4 * H],
                                 start=False, stop=True)
                nc.vector.tensor_tensor(Bn_sb[:, :], rz_sb[:, 0:H], hn_ps[:, :],
                                        op=ALU.mult)
                nc.vector.tensor_tensor(cAB_sb[:, :], gn_ps[:, :], Bn_sb[:, :],
                                        op=ALU.add)
                nc.scalar.activation(cAB_sb[:, :], cAB_sb[:, :], AF.Tanh)
                # h' = n + z * (h - n)
                nc.vector.tensor_tensor(Bn_sb[:, :], h_bh[:, :], cAB_sb[:, :],
                                        op=ALU.subtract)
                nc.vector.tensor_tensor(Bn_sb[:, :], rz_sb[:, H:], Bn_sb[:, :],
                                        op=ALU.mult)
                nc.vector.tensor_tensor(h_bh[:, :], cAB_sb[:, :], Bn_sb[:, :],
                                        op=ALU.add)

                # h^T into history (PE transpose per h-chunk)
                for hc in range(HC):
                    tr_ps = psSm.tile([128, B], BF16, tag="sm")
                    nc.tensor.transpose(
                        tr_ps[:, :], h_bh[:, hc * 128:(hc + 1) * 128],
                        ident_sb[:, :])
                    nc.vector.tensor_copy(hist[:, hc, k % 8, :], tr_ps[:, :])

            # ---- generator tail: rows for steps 24..25 ----
            for gi, (k0, nk, off, width) in enumerate([] if "gen" in skip else GEN_TAIL):
                m = B * nk
                bge = gop.tile([1, NSUB], BF16, tag="bg", bufs=4)
                nc.sync.dma_start(bge[:, 0:width], bgen[:, off:off + width])
                o_ps = psO.tile([128, NSUB], F32, tag="gen")
                s0 = k0 % 8
                for hc in range(HC):
                    nc.tensor.matmul(
                        o_ps[:m, 0:width],
                        hist[:, hc, s0:s0 + nk, :],
                        wgen_sb[:, hc, off:off + width],
                        start=(hc == 0), stop=False)
                nc.tensor.matmul(
                    o_ps[:m, 0:width], ones1b_sb[0:1, 0:m], bge[0:1, 0:width],
                    start=False, stop=True)
                o_sb = gop.tile([128, NSUB], F32, tag="ob")
                if gi % 2 == 0:
                    nc.vector.tensor_copy(o_sb[:m, 0:width], o_ps[:m, 0:width])
                else:
                    nc.scalar.activation(o_sb[:m, 0:width], o_ps[:m, 0:width],
                                         AF.Copy)
                nc.sync.dma_start(
                    out[k0:k0 + nk, :, off:off + width], o_sb[:m, 0:width])

    nc.finalize()
    return nc


def _get_graph():
    if not hasattr(_get_graph, "_nc"):
        _get_graph._nc = build_graph()
    return _get_graph._nc


def make_in_maps(feats, text_length, W_i2h, W_h2h, b_h2h, W_score, W_ih, W_hh,
                 b_ih, b_hh, W_gen, b_gen):
    bf = ml_dtypes.bfloat16
    feats = np.asarray(feats, np.float32)

    wi2h = np.ascontiguousarray(np.asarray(W_i2h, np.float32).T).astype(bf)
    wh2h = np.ascontiguousarray(np.asarray(W_h2h, np.float32).T).astype(bf)
    bh2h = np.ascontiguousarray(
        np.asarray(b_h2h, np.float32).reshape(HC, 128).T)
    wcol = np.ascontiguousarray(
        np.asarray(W_score, np.float32)[0].reshape(HC, 128).T).astype(bf)
    wih = np.ascontiguousarray(np.asarray(W_ih, np.float32).T).astype(bf)
    whh = np.ascontiguousarray(np.asarray(W_hh, np.float32).T).astype(bf)
    b_ih = np.asarray(b_ih, np.float32)
    b_hh = np.asarray(b_hh, np.float32)
    brow1 = np.concatenate([b_ih[:2 * H] + b_hh[:2 * H],
                            b_ih[2 * H:], b_hh[2 * H:]])
    brow = brow1[None, :].astype(bf)
    wgen = np.ascontiguousarray(np.asarray(W_gen, np.float32).T).astype(bf)
    bgen = np.asarray(b_gen, np.float32)[None, :].astype(bf)
    ident = np.eye(128, dtype=np.float32).astype(bf)

    in_maps = []
    for c in range(NCORES):
        fsh = feats[:, c * B:(c + 1) * B, :]                     # [T, B, C]
        featsT = np.ascontiguousarray(fsh.transpose(2, 0, 1)).astype(bf)
        featsS = np.ascontiguousarray(fsh.transpose(1, 0, 2)).astype(bf)
        in_maps.append({
            "featsT": featsT, "featsS": featsS, "wi2h": wi2h, "wh2h": wh2h,
            "bh2h": bh2h, "wcol": wcol, "wih": wih, "whh": whh, "brow": brow,
            "wgen": wgen, "bgen": bgen, "ident": ident,
        })

    return in_maps


def kernel(**inputs):
    nc = _get_graph()
    in_maps = make_in_maps(**inputs)
    res = run_bass_kernel_spmd(nc, in_maps, core_ids=list(range(NCORES)))
    return np.concatenate(
        [res.results[c]["out"].transpose(1, 0, 2).reshape(B * L, NCLS)
         for c in range(NCORES)],
        axis=0)



# revision 23
# speedup vs baseline: 170.5849x; 170.5849x over previous
"""Bass/Trainium2 kernel for the attention-decoder problem (v3.1).

Data-parallel over batch: 8 cores x 32 batch each. Per core, a 26-step
Bahdanau-attention + GRU scan over T=128 encoder positions, then a
generator GEMM [832, 512] @ [512, 6736].

Structure:
- 2-way software pipeline over half-batches (16+16): half B runs half a
  step behind half A, so B's attention z-phase (Act-engine tanh, the
  per-step floor) overlaps A's serial tail (softmax, context, GRU) and
  vice versa; the Act engine stays ~continuously busy.
- gates are computed TRANSPOSED: stationary = W 128x128 chunks, moving =
  ctx^T/h^T [128, 16]; preactivations land [3H | b] in PSUM, gate math
  runs fully partition-parallel, and h' lands directly in [h | b] -- no
  transposes anywhere. The W_hh half of the gate GEMM only needs
  h(k-1), so it is issued at the start of the previous covering phase,
  off the critical chain; the W_ih half follows the context.
- context is computed TRANSPOSED: per (b, c-chunk) one matmul with
  stationary = feats_b chunk [t, c] and moving = the UNNORMALIZED
  exp(e) column; 1/s is broadcast across partitions via a k=1 matmul
  (computed in parallel with the context matmuls) and folded into the
  context PSUM->SBUF evacuation multiply.
- e^T accumulates per z-chunk into 4 PSUM columns right after each
  tanh; partial sums run early so only the last chunk's dot + one add
  sit on the critical chain.
- sigmoid(x) = 0.5 tanh(x/2) + 0.5 is algebraically folded so no gate
  fix-up op sits on the chain: with A' = 2*(W_ih,n ctx + b_ih,n)
  (weights pre-doubled on the host), n = tanh(0.5*(A' + (t_r+1)*gh_n))
  and h' = n + 0.5*(t_z+1)*(h-n), via scalar_tensor_tensor.
- all biases ride into PSUM as k=1 matmuls with stationary = bias-row.
- generator PSUM->SBUF copies run on the Act engine, queued exactly in
  the inter-phase boundary gap (while DVE runs the next z-add).
"""

import numpy as np
import ml_dtypes

import concourse.bass as bass
import concourse.mybir as mybir
import concourse.tile as tile
from concourse import bacc
from concourse.bass_utils import run_bass_kernel_spmd

BF16 = mybir.dt.bfloat16
F32 = mybir.dt.float32
AF = mybir.ActivationFunctionType
ALU = mybir.AluOpType

T, BFULL, C = 128, 256, 512
H, L, NCLS = 512, 26, 6736
NCORES = 8
B = BFULL // NCORES          # 32 per core
BH = B // 2                  # 16 per pipeline half
HC = H // 128                # 4 h-chunks
CC = C // 128                # 4 c-chunks
G3 = 3 * H                   # 1536
NSUB = 512                   # class-chunk width (one PSUM bank of f32)
NSUBS = [(i * NSUB, min(NSUB, NCLS - i * NSUB))
         for i in range((NCLS + NSUB - 1) // NSUB)]     # 14 chunks


def _gen_schedule():
    """sched[k] = list of (k0, nk, off, width) generator sub-GEMMs emitted
    during scan step k (group g: steps [4g, 4g+4), emitted over steps
    4g+4..4g+7, or 24..25 for g=5; g=6 runs after the scan)."""
    sched = {k: [] for k in range(L)}
    for g in range(6):
        k0, nk = 4 * g, 4
        if g < 5:
            # skip the first eligible step entirely: the other half's h for
            # the group's last step lands mid-phase there, and gen matmuls
            # waiting on it cascade into Act head-of-line stalls
            steps = [4 * g + 5] * 6 + [4 * g + 6] * 4 + [4 * g + 7] * 4
        else:
            steps = [24] * 7 + [25] * 7
        for i, (off, width) in enumerate(NSUBS):
            sched[steps[i]].append((k0, nk, off, width))
    return sched

GEN_SCHED = _gen_schedule()
GEN_TAIL = [(24, 2, off, width) for (off, width) in NSUBS]


def build_graph(reps=1, skip=()):
    nc = bacc.Bacc(None, target_bir_lowering=False, debug=False)

    # ---- DRAM parameters (per-core shard shapes) ----
    featsT = nc.declare_dram_parameter("featsT", [C, B, T], BF16, isOutput=False)
    featsS = nc.declare_dram_parameter("featsS", [B, T, C], BF16, isOutput=False)
    wi2h = nc.declare_dram_parameter("wi2h", [C, H], BF16, isOutput=False)
    wh2h = nc.declare_dram_parameter("wh2h", [H, H], BF16, isOutput=False)
    bh2hr = nc.declare_dram_parameter("bh2hr", [1, H], BF16, isOutput=False)
    wcol = nc.declare_dram_parameter("wcol", [128, HC], BF16, isOutput=False)
    wih = nc.declare_dram_parameter("wih", [C, G3], BF16, isOutput=False)
    whh = nc.declare_dram_parameter("whh", [H, G3], BF16, isOutput=False)
    brow = nc.declare_dram_parameter("brow", [1, 4 * H], BF16, isOutput=False)
    wgen = nc.declare_dram_parameter("wgen", [H, NCLS], BF16, isOutput=False)
    bgen = nc.declare_dram_parameter("bgen", [1, NCLS], BF16, isOutput=False)
    out = nc.declare_dram_parameter("out", [L, B, NCLS], F32, isOutput=True)

    with tile.TileContext(nc) as tc:
        with tc.tile_pool(name="pp", bufs=1) as pp:
            # ---- persistent SBUF ----
            P_sb = pp.tile([128, HC, T, B], BF16)      # i2h proj, [h | t, b]
            fs_sb = pp.tile([128, B, C], BF16)         # feats [t | b, c]
            wh2h_sb = pp.tile([128, HC, H], BF16)
            bh2hr_sb = pp.tile([1, H], BF16)
            wcol_sb = pp.tile([128, HC], BF16)
            wih_sb = pp.tile([128, CC, G3], BF16)
            whh_sb = pp.tile([128, HC, G3], BF16)
            brow_sb = pp.tile([1, 4 * H], BF16)
            wgen_sb = pp.tile([128, HC, NCLS], BF16)
            # h^T ring buffer, slot-major: slot k%16 holds h_k^T [h | b].
            # 16-deep so generator reads of old slots never make the h'
            # write wait (WAR hazard with a lagging gen group).
            hist = pp.tile([128, HC, 16, B], BF16)
            hp_sb = pp.tile([128, HC, B], BF16)        # hp^T evac
            ctxT_sb = pp.tile([128, CC, B], BF16)      # ctx^T evac
            et_sb = pp.tile([128, B], F32)             # e^T running sum
            E_sb = pp.tile([128, B], BF16)             # exp(e^T), unnormalized
            recip_sb = pp.tile([1, B], BF16)           # 1/s
            bc_sb = pp.tile([128, 1, B], F32)          # 1/s bcast across t
            g2_sb = pp.tile([128, 8, B], F32)          # rz pre-act (hh+ih)
            rz_sb = pp.tile([128, 8, B], BF16)         # raw tanh(0.5 rz)
            n_sb = pp.tile([128, HC, B], F32)          # n gate
            t1_sb = pp.tile([128, HC, B], F32)         # gate temp
            ones128_sb = pp.tile([128, 1], BF16)
            ones1b_sb = pp.tile([1, 128], BF16)
            bgen_sb = pp.tile([1, NCLS], BF16)

            # ---- constant / weight loads, spread over 4 DMA rings so
            #      descriptor issue (~0.6us per dma_start per ring) overlaps ----
            nc.vector.memset(ones128_sb[:, :], 1.0)
            nc.vector.memset(ones1b_sb[:, :], 1.0)
            nc.vector.memset(hist[:, :, 15, :], 0.0)  # h_{-1} = 0 at slot 15
            for kc in range(HC):   # first use: hp at scan start
                nc.gpsimd.dma_start(wh2h_sb[:, kc, :],
                                    wh2h[kc * 128:(kc + 1) * 128, :])
            nc.gpsimd.dma_start(bh2hr_sb[:, :], bh2hr[:, :])
            nc.gpsimd.dma_start(wcol_sb[:, :], wcol[:, :])
            nc.gpsimd.dma_start(brow_sb[:, :], brow[:, :])
            for b0 in range(0, B, 4):  # first use: ctx, ~1 phase into scan
                nc.gpsimd.dma_start(
                    fs_sb[:, b0:b0 + 4, :],
                    featsS[b0:b0 + 4, :, :].rearrange("b t c -> t b c"))
            nc.gpsimd.dma_start(bgen_sb[:, :], bgen[:, :])

            # ---- prologue: P^T = W_i2h^T @ feats^T, laid out [h | t, b] ----
            with (
                tc.tile_pool(name="ftp", bufs=2) as fp,
                tc.tile_pool(name="psP", bufs=4, space="PSUM") as psP,
            ):
                wi2h_sb = fp.tile([128, CC, H], BF16, tag="w", bufs=1)
                for kc in range(CC):
                    nc.scalar.dma_start(wi2h_sb[:, kc, :],
                                        wi2h[kc * 128:(kc + 1) * 128, :])
                # iterate over 4-sample groups (b-major) so half A's P is
                # complete at the halfway point and the scan starts early
                for bs in range(8):
                    ft_bs = fp.tile([128, CC, 4, T], BF16, tag="ft", bufs=3)
                    for kc in range(CC):
                        ring = nc.sync if kc % 2 == 0 else nc.scalar
                        ring.dma_start(
                            ft_bs[:, kc, :, :],
                            featsT[kc * 128:(kc + 1) * 128,
                                   bs * 4:(bs + 1) * 4, :])
                    for mp in range(0, HC, 2):
                        pps = [psP.tile([128, 512], F32, tag="pp",
                                        name=f"ppsum{q}") for q in (mp, mp + 1)]
                        for kc in range(CC):
                            for mc, ppsum in zip((mp, mp + 1), pps):
                                nc.tensor.matmul(
                                    ppsum[:, :],
                                    wi2h_sb[:, kc, mc * 128:(mc + 1) * 128],
                                    ft_bs[:, kc, :, :],
                                    start=(kc == 0), stop=(kc == CC - 1))
                        for mc, ppsum in zip((mp, mp + 1), pps):
                            dst = P_sb[:, mc, :, bs * 4:(bs + 1) * 4]
                            src_ap = ppsum[:, :].rearrange(
                                "p (b t) -> p t b", b=4)
                            if mc % 2 == 0:
                                nc.vector.tensor_copy(dst, src_ap)
                            else:
                                nc.scalar.activation(dst, src_ap, AF.Copy)

            state = [None, None]   # per-half carry between phases
            genct = [0]            # alternate gen-copy engine

            def emit_hp(h0, k):
                """hp^T = W_h2h^T h^T + b -> PSUM; evac chunk 0 first so the
                first z-add can start before the rest lands."""
                hp_ps = psS.tile([128, HC, BH], F32, tag="sm")
                for mc in range(HC):
                    for kc in range(HC):
                        nc.tensor.matmul(
                            hp_ps[:, mc, :],
                            wh2h_sb[:, kc, mc * 128:(mc + 1) * 128],
                            hist[:, kc, (k - 1) % 16, h0:h0 + BH],
                            start=(kc == 0), stop=False)
                    nc.tensor.matmul(
                        hp_ps[:, mc, :],
                        bh2hr_sb[0:1, mc * 128:(mc + 1) * 128],
                        ones1b_sb[0:1, 0:BH], start=False, stop=True)
                nc.vector.tensor_copy(hp_sb[:, 0, h0:h0 + BH], hp_ps[:, 0, :])
                nc.vector.tensor_copy(hp_sb[:, 1:, h0:h0 + BH], hp_ps[:, 1:, :])

            def emit_gates_hh(h0, k):
                """W_hh part of the gate GEMM + n_h bias; only needs h(k-1),
                so it runs at the covering phase's start, off-chain.
                g tile regions: 0-7 rz(hh), 8-11 n_h(+b_hh,n),
                12-19 rz(ih)+b_rz, 20-23 2*(n_i+b_ih,n)."""
                g_ps = psG.tile([128, 24, BH], F32, tag="g")
                state[h0 // BH]["g"] = g_ps
                if "gates" in skip:
                    return
                hT = hist[:, :, (k - 1) % 16, :]
                for mi in range(12):
                    col = (mi % 4) * 128 + (0 if mi < 4 else
                                            512 if mi < 8 else 1024)
                    for kc in range(HC):
                        nc.tensor.matmul(
                            g_ps[:, mi, :], whh_sb[:, kc, col:col + 128],
                            hT[:, kc, h0:h0 + BH],
                            start=(kc == 0), stop=(mi < 8 and kc == HC - 1))
                    if mi >= 8:   # + b_hh,n
                        boff = 1536 + (mi - 8) * 128
                        nc.tensor.matmul(
                            g_ps[:, mi, :], brow_sb[0:1, boff:boff + 128],
                            ones1b_sb[0:1, 0:BH], start=False, stop=True)

            def emit_z(h0, hc, split=False):
                z = zp.tile([128, T, BH], BF16, tag="z")
                spans = ((0, T // 2), (T // 2, T)) if split else ((0, T),)
                for (t0, t1) in spans:
                    nc.vector.tensor_tensor(
                        z[:, t0:t1, :], P_sb[:, hc, t0:t1, h0:h0 + BH],
                        hp_sb[:, hc:hc + 1, h0:h0 + BH]
                        .broadcast_to((128, t1 - t0, BH)),
                        op=ALU.add)
                    if "tanh" not in skip:
                        nc.scalar.activation(z[:, t0:t1, :], z[:, t0:t1, :],
                                             AF.Tanh)
                return z

            def emit_e(z, hc, e_ps, h0, half=None):
                """one e^T column set from z chunk hc + running et update.
                half=0/1 restricts to t rows [0,64)/[64,128) (chunk-3 split
                so the first half overlaps the second half's tanh)."""
                if "e" in skip:
                    if hc == 0:
                        nc.vector.memset(et_sb[:, h0:h0 + BH], 0.5)
                    return
                r0, r1 = (0, T) if half is None else (half * 64, half * 64 + 64)
                for b in range(BH):
                    nc.tensor.matmul(
                        e_ps[r0:r1, hc, b:b + 1], z[:, r0:r1, b],
                        wcol_sb[:, hc:hc + 1], start=True, stop=True)
                if hc == 0:
                    nc.vector.tensor_copy(et_sb[:, h0:h0 + BH], e_ps[:, 0, :])
                elif hc == 1:
                    nc.vector.tensor_tensor(et_sb[:, h0:h0 + BH],
                                            et_sb[:, h0:h0 + BH],
                                            e_ps[:, 1, :], op=ALU.add)
                elif hc == 2:
                    nc.vector.tensor_tensor(et_sb[:, h0:h0 + BH],
                                            et_sb[:, h0:h0 + BH],
                                            e_ps[:, 2, :], op=ALU.add)
                elif hc == 3:
                    nc.vector.tensor_tensor(et_sb[r0:r1, h0:h0 + BH],
                                            et_sb[r0:r1, h0:h0 + BH],
                                            e_ps[r0:r1, 3, :], op=ALU.add)

            def emit_gen_mm(k0, nk, off, width):
                m = B * nk
                s0 = k0 % 16
                o_ps = psO.tile([128, NSUB], F32, tag="gen")
                for hc in range(HC):
                    nc.tensor.matmul(
                        o_ps[:m, 0:width],
                        hist[:, hc, s0:s0 + nk, :],
                        wgen_sb[:, hc, off:off + width],
                        start=(hc == 0), stop=False)
                nc.tensor.matmul(
                    o_ps[:m, 0:width], ones1b_sb[0:1, 0:m],
                    bgen_sb[0:1, off:off + width], start=False, stop=True)
                return o_ps, m

            def emit_gen_out(o_ps, m, k0, nk, off, width, on_act=True,
                             ring=None):
                o_sb = gop.tile([128, NSUB], F32, tag="ob")
                if on_act:
                    nc.scalar.activation(o_sb[:m, 0:width], o_ps[:m, 0:width],
                                         AF.Copy)
                else:
                    nc.vector.tensor_copy(o_sb[:m, 0:width], o_ps[:m, 0:width])
                if ring is None:
                    genct[0] += 1
                    ring = nc.sync if genct[0] % 2 == 0 else nc.scalar
                ring.dma_start(
                    out[k0:k0 + nk, :, off:off + width], o_sb[:m, 0:width])

            # ---- tail sub-blocks for half at h0 (its z-phase ran last
            #      phase; e_ps columns 0-2 are already summed in et_sb) ----
            def tail_softmax(h0):
                st = state[h0 // BH]
                nc.scalar.activation(E_sb[:, h0:h0 + BH], et_sb[:, h0:h0 + BH],
                                     AF.Exp)
                s_ps = psS.tile([1, BH], F32, tag="sm")
                nc.tensor.matmul(s_ps[:, :], ones128_sb[:, :],
                                 E_sb[:, h0:h0 + BH], start=True, stop=True)
                with nc.allow_low_precision(reason="1/s bf16; ctx is bf16"):
                    nc.vector.reciprocal(recip_sb[:, h0:h0 + BH], s_ps[:, :])
                bc_ps = psS.tile([128, 1, BH], F32, tag="sm")
                nc.tensor.matmul(bc_ps[:, 0, :], ones1b_sb[0:1, :],
                                 recip_sb[:, h0:h0 + BH], start=True, stop=True)
                nc.vector.tensor_copy(bc_sb[:, :, h0:h0 + BH], bc_ps[:, :, :])

            def tail_ctx(h0):
                st = state[h0 // BH]
                if "ctx" in skip:
                    nc.vector.memset(ctxT_sb[:, :, h0:h0 + BH], 0.01)
                    return
                ctx_ps = psS.tile([128, CC, BH], F32, tag="sm")
                for b in range(BH):
                    for cc in range(CC):
                        nc.tensor.matmul(
                            ctx_ps[:, cc, b:b + 1],
                            fs_sb[:, h0 + b, cc * 128:(cc + 1) * 128],
                            E_sb[:, h0 + b:h0 + b + 1],
                            start=True, stop=True)
                # evac with the 1/s scale folded in
                nc.vector.tensor_tensor(
                    ctxT_sb[:, :, h0:h0 + BH], ctx_ps[:, :, :],
                    bc_sb[:, :, h0:h0 + BH].broadcast_to((128, CC, BH)),
                    op=ALU.mult)

            def tail_gates_ih(h0):
                if "gates" in skip:
                    return
                g_ps = state[h0 // BH]["g"]
                for mi in range(12, 24):
                    col = ((mi - 12) % 4) * 128 + (0 if mi < 16 else
                                                   512 if mi < 20 else 1024)
                    for kc in range(CC):
                        nc.tensor.matmul(
                            g_ps[:, mi, :], wih_sb[:, kc, col:col + 128],
                            ctxT_sb[:, kc, h0:h0 + BH],
                            start=(kc == 0), stop=False)
                    boff = (mi - 12) * 128 if mi < 20 else 1024 + (mi - 20) * 128
                    nc.tensor.matmul(
                        g_ps[:, mi, :], brow_sb[0:1, boff:boff + 128],
                        ones1b_sb[0:1, 0:BH], start=False, stop=True)
                # rz preactivation = hh part + ih part (the hh evac runs
                # early, off-chain: its matmuls finished at phase start)
                nc.vector.tensor_copy(g2_sb[:, :, h0:h0 + BH], g_ps[:, 0:8, :])
                nc.vector.tensor_tensor(g2_sb[:, :, h0:h0 + BH],
                                        g2_sb[:, :, h0:h0 + BH],
                                        g_ps[:, 12:20, :], op=ALU.add)
                nc.scalar.activation(rz_sb[:, :, h0:h0 + BH],
                                     g2_sb[:, :, h0:h0 + BH], AF.Tanh,
                                     scale=0.5)

            def tail_n_h(h0, k):
                if "gates" in skip:
                    nc.vector.memset(hist[:, :, k % 16, h0:h0 + BH], 0.01)
                    return
                g_ps = state[h0 // BH]["g"]
                hprev = hist[:, :, (k - 1) % 16, h0:h0 + BH]
                t1 = t1_sb[:, :, h0:h0 + BH]
                n = n_sb[:, :, h0:h0 + BH]
                # n = tanh(0.5*(A' + (t_r+1)*gh_n)), A' = 2*(W_ih,n ctx + b)
                nc.vector.scalar_tensor_tensor(
                    t1, rz_sb[:, 0:4, h0:h0 + BH], 1.0, g_ps[:, 8:12, :],
                    ALU.add, ALU.mult)
                nc.vector.tensor_tensor(t1, t1, g_ps[:, 20:24, :], op=ALU.add)
                nc.scalar.activation(n, t1, AF.Tanh, scale=0.5)
                # h' = n + 0.5*(t_z+1)*(h-n)
                nc.vector.tensor_tensor(t1, hprev, n, op=ALU.subtract)
                nc.vector.scalar_tensor_tensor(
                    t1, rz_sb[:, 4:8, h0:h0 + BH], 1.0, t1, ALU.add, ALU.mult)
                nc.vector.scalar_tensor_tensor(
                    hist[:, :, k % 16, h0:h0 + BH], t1, 0.5, n,
                    ALU.mult, ALU.add)

            def emit_phase(h0, k, tail_h0, tail_k, gen_items):
                """z-phase for half h0 at step k, with the other half's tail
                (for step tail_k) interleaved at fixed points."""
                me = state[h0 // BH] = {}
                do_tail = tail_k >= 0 and "tail" not in skip
                emit_hp(h0, k)
                emit_gates_hh(h0, k)
                e_ps = psS.tile([128, HC, BH], F32, tag="e", bufs=1)
                me["e"] = e_ps
                zts = [emit_z(h0, 0, split=True)]
                if do_tail:
                    tail_softmax(tail_h0)
                zts.append(emit_z(h0, 1))
                if do_tail:
                    tail_ctx(tail_h0)
                    tail_gates_ih(tail_h0)
                zts.append(emit_z(h0, 2))
                emit_e(zts[0], 0, e_ps, h0)
                emit_e(zts[1], 1, e_ps, h0)
                if do_tail:
                    tail_n_h(tail_h0, tail_k)
                zts.append(emit_z(h0, 3, split=True))
                emit_e(zts[2], 2, e_ps, h0)
                gen_work = [emit_gen_mm(*gi) + gi for gi in gen_items]
                emit_e(zts[3], 3, e_ps, h0, half=0)
                emit_e(zts[3], 3, e_ps, h0, half=1)
                # gen evacuations land in the boundary gap while DVE runs
                # the next phase's first z-add
                for (o_ps, m, k0, nk, off, width) in gen_work:
                    emit_gen_out(o_ps, m, k0, nk, off, width)

            def emit_tail(h0, k):
                tail_softmax(h0)
                tail_ctx(h0)
                tail_gates_ih(h0)
                tail_n_h(h0, k)

            from contextlib import ExitStack
            for rep in range(reps):
                if rep > 0:
                    nc.vector.memset(hist[:, :, 15, :], 0.0)
                state[0] = {}
                state[1] = {}
                with ExitStack() as scan_pools:
                    zp = scan_pools.enter_context(
                        tc.tile_pool(name=f"zp{rep}", bufs=6))
                    gop = scan_pools.enter_context(
                        tc.tile_pool(name=f"gop{rep}", bufs=3))
                    psS = scan_pools.enter_context(
                        tc.tile_pool(name=f"psS{rep}", bufs=2, space="PSUM"))
                    psG = scan_pools.enter_context(
                        tc.tile_pool(name=f"psG{rep}", bufs=2, space="PSUM"))
                    psO = scan_pools.enter_context(
                        tc.tile_pool(name=f"psO{rep}", bufs=3, space="PSUM"))
                    for k in range(L):
                        if rep == 0 and k == 0:
                            # gate weights land behind the prologue's feats
                            # traffic; first use is the k=0 tails
                            for kc in range(HC):
                                nc.gpsimd.dma_start(
                                    whh_sb[:, kc, :],
                                    whh[kc * 128:(kc + 1) * 128, :])
                            for kc in range(CC):
                                nc.gpsimd.dma_start(
                                    wih_sb[:, kc, :],
                                    wih[kc * 128:(kc + 1) * 128, :])
                        if rep == 0 and k == 1:
                            # wgen lands here: after the prologue's featsT
                            # traffic, well before first use at step 5
                            for kc in range(HC):
                                nc.gpsimd.dma_start(
                                    wgen_sb[:, kc, :],
                                    wgen[kc * 128:(kc + 1) * 128, :])
                        gi = [] if "gen" in skip else GEN_SCHED[k]
                        # a group's first emission step: the B half's h for
                        # the group's last step lands mid-A-phase; emitting
                        # its chunks in the A phase would head-of-line block
                        # PE behind that write, so they all go to the B phase
                        if any(k0 + nk == k for (k0, nk, _, _) in gi):
                            ga, gb = [], gi
                        else:
                            ga, gb = gi[:len(gi) // 2], gi[len(gi) // 2:]
                        emit_phase(0, k, BH, k - 1, ga)
                        emit_phase(BH, k, 0, k, gb)
                    if "tail" not in skip:
                        emit_tail(BH, L - 1)

                # ---- generator tail: rows for steps 24..25, on wide pools
                #      so the 14 chunks pipeline instead of serializing ----
                with (
                    tc.tile_pool(name=f"gopD{rep}", bufs=6) as gopD,
                    tc.tile_pool(name=f"psD{rep}", bufs=6, space="PSUM") as psD,
                ):
                    gop, psO = gopD, psD
                    for i, (k0, nk, off, width) in enumerate(
                            [] if "gen" in skip else GEN_TAIL):
                        o_ps, m = emit_gen_mm(k0, nk, off, width)
                        emit_gen_out(o_ps, m, k0, nk, off, width,
                                     on_act=(i % 2 == 0),
                                     ring=(nc.sync if i % 2 == 0 else
                                           nc.scalar))

    nc.finalize()
    return nc


def _get_graph():
    if not hasattr(_get_graph, "_nc"):
        _get_graph._nc = build_graph()
    return _get_graph._nc


def make_in_maps(feats, text_length, W_i2h, W_h2h, b_h2h, W_score, W_ih, W_hh,
                 b_ih, b_hh, W_gen, b_gen):
    bf = ml_dtypes.bfloat16
    feats = np.asarray(feats, np.float32)

    wi2h = np.ascontiguousarray(np.asarray(W_i2h, np.float32).T).astype(bf)
    wh2h = np.ascontiguousarray(np.asarray(W_h2h, np.float32).T).astype(bf)
    bh2hr = np.asarray(b_h2h, np.float32)[None, :].astype(bf)
    wcol = np.ascontiguousarray(
        np.asarray(W_score, np.float32)[0].reshape(HC, 128).T).astype(bf)
    # n-gate input weights/bias doubled: n = tanh(0.5*(A' + (t_r+1)*gh_n))
    wih_f = np.ascontiguousarray(np.asarray(W_ih, np.float32).T).copy()
    wih_f[:, 2 * H:] *= 2.0
    wih = wih_f.astype(bf)
    whh = np.ascontiguousarray(np.asarray(W_hh, np.float32).T).astype(bf)
    b_ih = np.asarray(b_ih, np.float32)
    b_hh = np.asarray(b_hh, np.float32)
    brow1 = np.concatenate([b_ih[:2 * H] + b_hh[:2 * H],
                            2.0 * b_ih[2 * H:], b_hh[2 * H:]])
    brow = brow1[None, :].astype(bf)
    wgen = np.ascontiguousarray(np.asarray(W_gen, np.float32).T).astype(bf)
    bgen = np.asarray(b_gen, np.float32)[None, :].astype(bf)

    in_maps = []
    for c in range(NCORES):
        fsh = feats[:, c * B:(c + 1) * B, :]                     # [T, B, C]
        featsT = np.ascontiguousarray(fsh.transpose(2, 1, 0)).astype(bf)
        featsS = np.ascontiguousarray(fsh.transpose(1, 0, 2)).astype(bf)
        in_maps.append({
            "featsT": featsT, "featsS": featsS, "wi2h": wi2h, "wh2h": wh2h,
            "bh2hr": bh2hr, "wcol": wcol, "wih": wih, "whh": whh,
            "brow": brow, "wgen": wgen, "bgen": bgen,
        })

    return in_maps


def kernel(**inputs):
    nc = _get_graph()
    in_maps = make_in_maps(**inputs)
    res = run_bass_kernel_spmd(nc, in_maps, core_ids=list(range(NCORES)))
    return np.concatenate(
        [res.results[c]["out"].transpose(1, 0, 2).reshape(B * L, NCLS)
         for c in range(NCORES)],
        axis=0)


# revision 24
# speedup vs baseline: 173.5349x; 1.0173x over previous
"""Bass/Trainium2 kernel for the attention-decoder problem (v3.1).

Data-parallel over batch: 8 cores x 32 batch each. Per core, a 26-step
Bahdanau-attention + GRU scan over T=128 encoder positions, then a
generator GEMM [832, 512] @ [512, 6736].

Structure:
- 2-way software pipeline over half-batches (16+16): half B runs half a
  step behind half A, so B's attention z-phase (Act-engine tanh, the
  per-step floor) overlaps A's serial tail (softmax, context, GRU) and
  vice versa; the Act engine stays ~continuously busy.
- gates are computed TRANSPOSED: stationary = W 128x128 chunks, moving =
  ctx^T/h^T [128, 16]; preactivations land [3H | b] in PSUM, gate math
  runs fully partition-parallel, and h' lands directly in [h | b] -- no
  transposes anywhere. The W_hh half of the gate GEMM only needs
  h(k-1), so it is issued at the start of the previous covering phase,
  off the critical chain; the W_ih half follows the context.
- context is computed TRANSPOSED: per (b, c-chunk) one matmul with
  stationary = feats_b chunk [t, c] and moving = the UNNORMALIZED
  exp(e) column; 1/s is broadcast across partitions via a k=1 matmul
  (computed in parallel with the context matmuls) and folded into the
  context PSUM->SBUF evacuation multiply.
- e^T accumulates per z-chunk into 4 PSUM columns right after each
  tanh; partial sums run early so only the last chunk's dot + one add
  sit on the critical chain.
- sigmoid(x) = 0.5 tanh(x/2) + 0.5 is algebraically folded so no gate
  fix-up op sits on the chain: with A' = 2*(W_ih,n ctx + b_ih,n)
  (weights pre-doubled on the host), n = tanh(0.5*(A' + (t_r+1)*gh_n))
  and h' = n + 0.5*(t_z+1)*(h-n), via scalar_tensor_tensor.
- all biases ride into PSUM as k=1 matmuls with stationary = bias-row.
- generator PSUM->SBUF copies run on the Act engine, queued exactly in
  the inter-phase boundary gap (while DVE runs the next z-add).
"""

import numpy as np
import ml_dtypes

import concourse.bass as bass
import concourse.mybir as mybir
import concourse.tile as tile
from concourse import bacc
from concourse.bass_utils import run_bass_kernel_spmd

BF16 = mybir.dt.bfloat16
F32 = mybir.dt.float32
AF = mybir.ActivationFunctionType
ALU = mybir.AluOpType

T, BFULL, C = 128, 256, 512
H, L, NCLS = 512, 26, 6736
NCORES = 8
B = BFULL // NCORES          # 32 per core
BH = B // 2                  # 16 per pipeline half
HC = H // 128                # 4 h-chunks
CC = C // 128                # 4 c-chunks
G3 = 3 * H                   # 1536
NSUB = 512                   # class-chunk width (one PSUM bank of f32)
NSUBS = [(i * NSUB, min(NSUB, NCLS - i * NSUB))
         for i in range((NCLS + NSUB - 1) // NSUB)]     # 14 chunks


def _gen_schedule():
    """sched[k] = list of (k0, nk, off, width) generator sub-GEMMs emitted
    during scan step k (group g: steps [4g, 4g+4), emitted over steps
    4g+4..4g+7, or 24..25 for g=5; g=6 runs after the scan)."""
    sched = {k: [] for k in range(L)}
    for g in range(6):
        k0, nk = 4 * g, 4
        if g < 5:
            # first step's chunks run in the B phase only (the other half's
            # h for the group's last step lands mid-A-phase); with the
            # 16-deep hist ring this is safe and keeps every boundary fed
            steps = [4 * g + 4] * 3 + [4 * g + 5] * 4 + [4 * g + 6] * 4 \
                + [4 * g + 7] * 3
        else:
            steps = [24] * 7 + [25] * 7
        for i, (off, width) in enumerate(NSUBS):
            sched[steps[i]].append((k0, nk, off, width))
    return sched

GEN_SCHED = _gen_schedule()
GEN_TAIL = [(24, 2, off, width) for (off, width) in NSUBS]


def build_graph(reps=1, skip=()):
    nc = bacc.Bacc(None, target_bir_lowering=False, debug=False)

    # ---- DRAM parameters (per-core shard shapes) ----
    featsT = nc.declare_dram_parameter("featsT", [C, B, T], BF16, isOutput=False)
    featsS = nc.declare_dram_parameter("featsS", [B, T, C], BF16, isOutput=False)
    wi2h = nc.declare_dram_parameter("wi2h", [C, H], BF16, isOutput=False)
    wh2h = nc.declare_dram_parameter("wh2h", [H, H], BF16, isOutput=False)
    bh2hr = nc.declare_dram_parameter("bh2hr", [1, H], BF16, isOutput=False)
    wcol = nc.declare_dram_parameter("wcol", [128, HC], BF16, isOutput=False)
    wih = nc.declare_dram_parameter("wih", [C, G3], BF16, isOutput=False)
    whh = nc.declare_dram_parameter("whh", [H, G3], BF16, isOutput=False)
    brow = nc.declare_dram_parameter("brow", [1, 4 * H], BF16, isOutput=False)
    wgen = nc.declare_dram_parameter("wgen", [H, NCLS], BF16, isOutput=False)
    bgen = nc.declare_dram_parameter("bgen", [1, NCLS], BF16, isOutput=False)
    out = nc.declare_dram_parameter("out", [L, B, NCLS], F32, isOutput=True)

    with tile.TileContext(nc) as tc:
        with tc.tile_pool(name="pp", bufs=1) as pp:
            # ---- persistent SBUF ----
            P_sb = pp.tile([128, HC, T, B], BF16)      # i2h proj, [h | t, b]
            fs_sb = pp.tile([128, B, C], BF16)         # feats [t | b, c]
            wh2h_sb = pp.tile([128, HC, H], BF16)
            bh2hr_sb = pp.tile([1, H], BF16)
            wcol_sb = pp.tile([128, HC], BF16)
            wih_sb = pp.tile([128, CC, G3], BF16)
            whh_sb = pp.tile([128, HC, G3], BF16)
            brow_sb = pp.tile([1, 4 * H], BF16)
            wgen_sb = pp.tile([128, HC, NCLS], BF16)
            # h^T ring buffer, slot-major: slot k%16 holds h_k^T [h | b].
            # 16-deep so generator reads of old slots never make the h'
            # write wait (WAR hazard with a lagging gen group).
            hist = pp.tile([128, HC, 16, B], BF16)
            hp_sb = pp.tile([128, HC, B], BF16)        # hp^T evac
            ctxT_sb = pp.tile([128, CC, B], BF16)      # ctx^T evac
            et_sb = pp.tile([128, B], F32)             # e^T running sum
            E_sb = pp.tile([128, B], BF16)             # exp(e^T), unnormalized
            recip_sb = pp.tile([1, B], BF16)           # 1/s
            bc_sb = pp.tile([128, 1, B], F32)          # 1/s bcast across t
            g2_sb = pp.tile([128, 8, B], F32)          # rz pre-act (hh+ih)
            rz_sb = pp.tile([128, 8, B], BF16)         # raw tanh(0.5 rz)
            n_sb = pp.tile([128, HC, B], F32)          # n gate
            t1_sb = pp.tile([128, HC, B], F32)         # gate temp
            ones128_sb = pp.tile([128, 1], BF16)
            ones1b_sb = pp.tile([1, 128], BF16)
            bgen_sb = pp.tile([1, NCLS], BF16)

            # ---- constant / weight loads, spread over 4 DMA rings so
            #      descriptor issue (~0.6us per dma_start per ring) overlaps ----
            nc.vector.memset(ones128_sb[:, :], 1.0)
            nc.vector.memset(ones1b_sb[:, :], 1.0)
            nc.vector.memset(hist[:, :, 15, :], 0.0)  # h_{-1} = 0 at slot 15
            for kc in range(HC):   # first use: hp at scan start
                nc.gpsimd.dma_start(wh2h_sb[:, kc, :],
                                    wh2h[kc * 128:(kc + 1) * 128, :])
            nc.gpsimd.dma_start(bh2hr_sb[:, :], bh2hr[:, :])
            nc.gpsimd.dma_start(wcol_sb[:, :], wcol[:, :])
            nc.gpsimd.dma_start(brow_sb[:, :], brow[:, :])
            nc.gpsimd.dma_start(bgen_sb[:, :], bgen[:, :])
            # big streams are scheduler-gated so their data transfers don't
            # contend with the prologue GEMM's featsT feed
            with tc.tile_wait_until(0.014):
                for b0 in range(0, B, 4):   # first use: ctx at ~60us
                    nc.gpsimd.dma_start(
                        fs_sb[:, b0:b0 + 4, :],
                        featsS[b0:b0 + 4, :, :].rearrange("b t c -> t b c"))
            with tc.tile_wait_until(0.030):  # first use ~58us (gates-hh k0)
                for kc in range(HC):
                    nc.gpsimd.dma_start(whh_sb[:, kc, :],
                                        whh[kc * 128:(kc + 1) * 128, :])
                for kc in range(CC):
                    nc.gpsimd.dma_start(wih_sb[:, kc, :],
                                        wih[kc * 128:(kc + 1) * 128, :])
            with tc.tile_wait_until(0.045):  # first use ~140us (gen step 4)
                for kc in range(HC):
                    nc.gpsimd.dma_start(wgen_sb[:, kc, :],
                                        wgen[kc * 128:(kc + 1) * 128, :])

            # ---- prologue: P^T = W_i2h^T @ feats^T, laid out [h | t, b] ----
            with (
                tc.tile_pool(name="ftp", bufs=2) as fp,
                tc.tile_pool(name="psP", bufs=4, space="PSUM") as psP,
            ):
                wi2h_sb = fp.tile([128, CC, H], BF16, tag="w", bufs=1)
                for kc in range(CC):
                    nc.scalar.dma_start(wi2h_sb[:, kc, :],
                                        wi2h[kc * 128:(kc + 1) * 128, :])
                # iterate over 4-sample groups (b-major) so half A's P is
                # complete at the halfway point and the scan starts early
                for bs in range(8):
                    ft_bs = fp.tile([128, CC, 4, T], BF16, tag="ft", bufs=5)
                    for kc in range(CC):
                        ring = nc.sync if kc % 2 == 0 else nc.scalar
                        ring.dma_start(
                            ft_bs[:, kc, :, :],
                            featsT[kc * 128:(kc + 1) * 128,
                                   bs * 4:(bs + 1) * 4, :])
                    for mp in range(0, HC, 2):
                        pps = [psP.tile([128, 512], F32, tag="pp",
                                        name=f"ppsum{q}") for q in (mp, mp + 1)]
                        for kc in range(CC):
                            for mc, ppsum in zip((mp, mp + 1), pps):
                                nc.tensor.matmul(
                                    ppsum[:, :],
                                    wi2h_sb[:, kc, mc * 128:(mc + 1) * 128],
                                    ft_bs[:, kc, :, :],
                                    start=(kc == 0), stop=(kc == CC - 1))
                        for mc, ppsum in zip((mp, mp + 1), pps):
                            dst = P_sb[:, mc, :, bs * 4:(bs + 1) * 4]
                            src_ap = ppsum[:, :].rearrange(
                                "p (b t) -> p t b", b=4)
                            if mc % 2 == 0:
                                nc.vector.tensor_copy(dst, src_ap)
                            else:
                                nc.scalar.activation(dst, src_ap, AF.Copy)

            state = [None, None]   # per-half carry between phases
            genct = [0]            # alternate gen-copy engine

            def emit_hp(h0, k):
                """hp^T = W_h2h^T h^T + b -> PSUM; evac chunk 0 first so the
                first z-add can start before the rest lands."""
                hp_ps = psS.tile([128, HC, BH], F32, tag="sm")
                for mc in range(HC):
                    for kc in range(HC):
                        nc.tensor.matmul(
                            hp_ps[:, mc, :],
                            wh2h_sb[:, kc, mc * 128:(mc + 1) * 128],
                            hist[:, kc, (k - 1) % 16, h0:h0 + BH],
                            start=(kc == 0), stop=False)
                    nc.tensor.matmul(
                        hp_ps[:, mc, :],
                        bh2hr_sb[0:1, mc * 128:(mc + 1) * 128],
                        ones1b_sb[0:1, 0:BH], start=False, stop=True)
                nc.vector.tensor_copy(hp_sb[:, 0, h0:h0 + BH], hp_ps[:, 0, :])
                nc.vector.tensor_copy(hp_sb[:, 1:, h0:h0 + BH], hp_ps[:, 1:, :])

            def emit_gates_hh(h0, k):
                """W_hh part of the gate GEMM + n_h bias; only needs h(k-1),
                so it runs at the covering phase's start, off-chain.
                g tile regions: 0-7 rz(hh), 8-11 n_h(+b_hh,n),
                12-19 rz(ih)+b_rz, 20-23 2*(n_i+b_ih,n)."""
                g_ps = psG.tile([128, 24, BH], F32, tag="g")
                state[h0 // BH]["g"] = g_ps
                if "gates" in skip:
                    return
                hT = hist[:, :, (k - 1) % 16, :]
                for mi in range(12):
                    col = (mi % 4) * 128 + (0 if mi < 4 else
                                            512 if mi < 8 else 1024)
                    for kc in range(HC):
                        nc.tensor.matmul(
                            g_ps[:, mi, :], whh_sb[:, kc, col:col + 128],
                            hT[:, kc, h0:h0 + BH],
                            start=(kc == 0), stop=(mi < 8 and kc == HC - 1))
                    if mi >= 8:   # + b_hh,n
                        boff = 1536 + (mi - 8) * 128
                        nc.tensor.matmul(
                            g_ps[:, mi, :], brow_sb[0:1, boff:boff + 128],
                            ones1b_sb[0:1, 0:BH], start=False, stop=True)

            def emit_z(h0, hc, split=False):
                z = zp.tile([128, T, BH], BF16, tag="z")
                spans = ((0, T // 2), (T // 2, T)) if split else ((0, T),)
                for (t0, t1) in spans:
                    nc.vector.tensor_tensor(
                        z[:, t0:t1, :], P_sb[:, hc, t0:t1, h0:h0 + BH],
                        hp_sb[:, hc:hc + 1, h0:h0 + BH]
                        .broadcast_to((128, t1 - t0, BH)),
                        op=ALU.add)
                    if "tanh" not in skip:
                        nc.scalar.activation(z[:, t0:t1, :], z[:, t0:t1, :],
                                             AF.Tanh)
                return z

            def emit_e(z, hc, e_ps, h0, half=None):
                """one e^T column set from z chunk hc + running et update.
                half=0/1 restricts to t rows [0,64)/[64,128) (chunk-3 split
                so the first half overlaps the second half's tanh)."""
                if "e" in skip:
                    if hc == 0:
                        nc.vector.memset(et_sb[:, h0:h0 + BH], 0.5)
                    return
                r0, r1 = (0, T) if half is None else (half * 64, half * 64 + 64)
                for b in range(BH):
                    nc.tensor.matmul(
                        e_ps[r0:r1, hc, b:b + 1], z[:, r0:r1, b],
                        wcol_sb[:, hc:hc + 1], start=True, stop=True)
                if hc == 0:
                    nc.vector.tensor_copy(et_sb[:, h0:h0 + BH], e_ps[:, 0, :])
                elif hc == 1:
                    nc.vector.tensor_tensor(et_sb[:, h0:h0 + BH],
                                            et_sb[:, h0:h0 + BH],
                                            e_ps[:, 1, :], op=ALU.add)
                elif hc == 2:
                    nc.vector.tensor_tensor(et_sb[:, h0:h0 + BH],
                                            et_sb[:, h0:h0 + BH],
                                            e_ps[:, 2, :], op=ALU.add)
                elif hc == 3:
                    nc.vector.tensor_tensor(et_sb[r0:r1, h0:h0 + BH],
                                            et_sb[r0:r1, h0:h0 + BH],
                                            e_ps[r0:r1, 3, :], op=ALU.add)

            def emit_gen_mm(k0, nk, off, width):
                m = B * nk
                s0 = k0 % 16
                o_ps = psO.tile([128, NSUB], F32, tag="gen")
                for hc in range(HC):
                    nc.tensor.matmul(
                        o_ps[:m, 0:width],
                        hist[:, hc, s0:s0 + nk, :],
                        wgen_sb[:, hc, off:off + width],
                        start=(hc == 0), stop=False)
                nc.tensor.matmul(
                    o_ps[:m, 0:width], ones1b_sb[0:1, 0:m],
                    bgen_sb[0:1, off:off + width], start=False, stop=True)
                return o_ps, m

            def emit_gen_out(o_ps, m, k0, nk, off, width, on_act=True,
                             ring=None):
                o_sb = gop.tile([128, NSUB], F32, tag="ob")
                if on_act:
                    nc.scalar.activation(o_sb[:m, 0:width], o_ps[:m, 0:width],
                                         AF.Copy)
                else:
                    nc.vector.tensor_copy(o_sb[:m, 0:width], o_ps[:m, 0:width])
                if ring is None:
                    genct[0] += 1
                    ring = nc.sync if genct[0] % 2 == 0 else nc.scalar
                ring.dma_start(
                    out[k0:k0 + nk, :, off:off + width], o_sb[:m, 0:width])

            # ---- tail sub-blocks for half at h0 (its z-phase ran last
            #      phase; e_ps columns 0-2 are already summed in et_sb) ----
            def tail_softmax(h0):
                st = state[h0 // BH]
                nc.scalar.activation(E_sb[:, h0:h0 + BH], et_sb[:, h0:h0 + BH],
                                     AF.Exp)
                s_ps = psS.tile([1, BH], F32, tag="sm")
                nc.tensor.matmul(s_ps[:, :], ones128_sb[:, :],
                                 E_sb[:, h0:h0 + BH], start=True, stop=True)
                with nc.allow_low_precision(reason="1/s bf16; ctx is bf16"):
                    nc.vector.reciprocal(recip_sb[:, h0:h0 + BH], s_ps[:, :])
                bc_ps = psS.tile([128, 1, BH], F32, tag="sm")
                nc.tensor.matmul(bc_ps[:, 0, :], ones1b_sb[0:1, :],
                                 recip_sb[:, h0:h0 + BH], start=True, stop=True)
                nc.vector.tensor_copy(bc_sb[:, :, h0:h0 + BH], bc_ps[:, :, :])

            def tail_ctx(h0):
                st = state[h0 // BH]
                if "ctx" in skip:
                    nc.vector.memset(ctxT_sb[:, :, h0:h0 + BH], 0.01)
                    return
                ctx_ps = psS.tile([128, CC, BH], F32, tag="sm")
                for b in range(BH):
                    for cc in range(CC):
                        nc.tensor.matmul(
                            ctx_ps[:, cc, b:b + 1],
                            fs_sb[:, h0 + b, cc * 128:(cc + 1) * 128],
                            E_sb[:, h0 + b:h0 + b + 1],
                            start=True, stop=True)
                # evac with the 1/s scale folded in
                nc.vector.tensor_tensor(
                    ctxT_sb[:, :, h0:h0 + BH], ctx_ps[:, :, :],
                    bc_sb[:, :, h0:h0 + BH].broadcast_to((128, CC, BH)),
                    op=ALU.mult)

            def tail_gates_ih(h0):
                if "gates" in skip:
                    return
                g_ps = state[h0 // BH]["g"]
                for mi in range(12, 24):
                    col = ((mi - 12) % 4) * 128 + (0 if mi < 16 else
                                                   512 if mi < 20 else 1024)
                    for kc in range(CC):
                        nc.tensor.matmul(
                            g_ps[:, mi, :], wih_sb[:, kc, col:col + 128],
                            ctxT_sb[:, kc, h0:h0 + BH],
                            start=(kc == 0), stop=False)
                    boff = (mi - 12) * 128 if mi < 20 else 1024 + (mi - 20) * 128
                    nc.tensor.matmul(
                        g_ps[:, mi, :], brow_sb[0:1, boff:boff + 128],
                        ones1b_sb[0:1, 0:BH], start=False, stop=True)
                # rz preactivation = hh part + ih part (the hh evac runs
                # early, off-chain: its matmuls finished at phase start)
                nc.vector.tensor_copy(g2_sb[:, :, h0:h0 + BH], g_ps[:, 0:8, :])
                nc.vector.tensor_tensor(g2_sb[:, :, h0:h0 + BH],
                                        g2_sb[:, :, h0:h0 + BH],
                                        g_ps[:, 12:20, :], op=ALU.add)
                nc.scalar.activation(rz_sb[:, :, h0:h0 + BH],
                                     g2_sb[:, :, h0:h0 + BH], AF.Tanh,
                                     scale=0.5)

            def tail_n_h(h0, k):
                if "gates" in skip:
                    nc.vector.memset(hist[:, :, k % 16, h0:h0 + BH], 0.01)
                    return
                g_ps = state[h0 // BH]["g"]
                hprev = hist[:, :, (k - 1) % 16, h0:h0 + BH]
                t1 = t1_sb[:, :, h0:h0 + BH]
                n = n_sb[:, :, h0:h0 + BH]
                # n = tanh(0.5*(A' + (t_r+1)*gh_n)), A' = 2*(W_ih,n ctx + b)
                nc.vector.scalar_tensor_tensor(
                    t1, rz_sb[:, 0:4, h0:h0 + BH], 1.0, g_ps[:, 8:12, :],
                    ALU.add, ALU.mult)
                nc.vector.tensor_tensor(t1, t1, g_ps[:, 20:24, :], op=ALU.add)
                nc.scalar.activation(n, t1, AF.Tanh, scale=0.5)
                # h' = n + 0.5*(t_z+1)*(h-n)
                nc.vector.tensor_tensor(t1, hprev, n, op=ALU.subtract)
                nc.vector.scalar_tensor_tensor(
                    t1, rz_sb[:, 4:8, h0:h0 + BH], 1.0, t1, ALU.add, ALU.mult)
                nc.vector.scalar_tensor_tensor(
                    hist[:, :, k % 16, h0:h0 + BH], t1, 0.5, n,
                    ALU.mult, ALU.add)

            def emit_phase(h0, k, tail_h0, tail_k, gen_items):
                """z-phase for half h0 at step k, with the other half's tail
                (for step tail_k) interleaved at fixed points."""
                me = state[h0 // BH] = {}
                do_tail = tail_k >= 0 and "tail" not in skip
                emit_hp(h0, k)
                emit_gates_hh(h0, k)
                e_ps = psS.tile([128, HC, BH], F32, tag="e", bufs=1)
                me["e"] = e_ps
                zts = [emit_z(h0, 0, split=True)]
                if do_tail:
                    tail_softmax(tail_h0)
                zts.append(emit_z(h0, 1))
                if do_tail:
                    tail_ctx(tail_h0)
                    tail_gates_ih(tail_h0)
                zts.append(emit_z(h0, 2))
                emit_e(zts[0], 0, e_ps, h0)
                emit_e(zts[1], 1, e_ps, h0)
                if do_tail:
                    tail_n_h(tail_h0, tail_k)
                zts.append(emit_z(h0, 3, split=True))
                emit_e(zts[2], 2, e_ps, h0)
                gen_work = [emit_gen_mm(*gi) + gi for gi in gen_items]
                emit_e(zts[3], 3, e_ps, h0, half=0)
                emit_e(zts[3], 3, e_ps, h0, half=1)
                # gen evacuations land in the boundary gap while DVE runs
                # the next phase's first z-add
                for (o_ps, m, k0, nk, off, width) in gen_work:
                    emit_gen_out(o_ps, m, k0, nk, off, width)

            def emit_tail(h0, k):
                tail_softmax(h0)
                tail_ctx(h0)
                tail_gates_ih(h0)
                tail_n_h(h0, k)

            from contextlib import ExitStack
            for rep in range(reps):
                if rep > 0:
                    nc.vector.memset(hist[:, :, 15, :], 0.0)
                state[0] = {}
                state[1] = {}
                with ExitStack() as scan_pools:
                    zp = scan_pools.enter_context(
                        tc.tile_pool(name=f"zp{rep}", bufs=6))
                    gop = scan_pools.enter_context(
                        tc.tile_pool(name=f"gop{rep}", bufs=3))
                    psS = scan_pools.enter_context(
                        tc.tile_pool(name=f"psS{rep}", bufs=2, space="PSUM"))
                    psG = scan_pools.enter_context(
                        tc.tile_pool(name=f"psG{rep}", bufs=2, space="PSUM"))
                    psO = scan_pools.enter_context(
                        tc.tile_pool(name=f"psO{rep}", bufs=3, space="PSUM"))
                    for k in range(L):
                        gi = [] if "gen" in skip else GEN_SCHED[k]
                        # a group's first emission step: the B half's h for
                        # the group's last step lands mid-A-phase; emitting
                        # its chunks in the A phase would head-of-line block
                        # PE behind that write, so they all go to the B phase
                        if any(k0 + nk == k for (k0, nk, _, _) in gi):
                            ga, gb = [], gi
                        else:
                            ga, gb = gi[:len(gi) // 2], gi[len(gi) // 2:]
                        emit_phase(0, k, BH, k - 1, ga)
                        emit_phase(BH, k, 0, k, gb)
                    if "tail" not in skip:
                        emit_tail(BH, L - 1)

                # ---- generator tail: rows for steps 24..25, on wide pools
                #      so the 14 chunks pipeline instead of serializing ----
                with (
                    tc.tile_pool(name=f"gopD{rep}", bufs=6) as gopD,
                    tc.tile_pool(name=f"psD{rep}", bufs=6, space="PSUM") as psD,
                ):
                    gop, psO = gopD, psD
                    for i, (k0, nk, off, width) in enumerate(
                            [] if "gen" in skip else GEN_TAIL):
                        o_ps, m = emit_gen_mm(k0, nk, off, width)
                        emit_gen_out(o_ps, m, k0, nk, off, width,
                                     on_act=(i % 2 == 0),
                                     ring=(nc.sync if i % 2 == 0 else
                                           nc.scalar))

    nc.finalize()
    return nc


def _get_graph():
    if not hasattr(_get_graph, "_nc"):
        _get_graph._nc = build_graph()
    return _get_graph._nc


def make_in_maps(feats, text_length, W_i2h, W_h2h, b_h2h, W_score, W_ih, W_hh,
                 b_ih, b_hh, W_gen, b_gen):
    bf = ml_dtypes.bfloat16
    feats = np.asarray(feats, np.float32)

    wi2h = np.ascontiguousarray(np.asarray(W_i2h, np.float32).T).astype(bf)
    wh2h = np.ascontiguousarray(np.asarray(W_h2h, np.float32).T).astype(bf)
    bh2hr = np.asarray(b_h2h, np.float32)[None, :].astype(bf)
    wcol = np.ascontiguousarray(
        np.asarray(W_score, np.float32)[0].reshape(HC, 128).T).astype(bf)
    # n-gate input weights/bias doubled: n = tanh(0.5*(A' + (t_r+1)*gh_n))
    wih_f = np.ascontiguousarray(np.asarray(W_ih, np.float32).T).copy()
    wih_f[:, 2 * H:] *= 2.0
    wih = wih_f.astype(bf)
    whh = np.ascontiguousarray(np.asarray(W_hh, np.float32).T).astype(bf)
    b_ih = np.asarray(b_ih, np.float32)
    b_hh = np.asarray(b_hh, np.float32)
    brow1 = np.concatenate([b_ih[:2 * H] + b_hh[:2 * H],
                            2.0 * b_ih[2 * H:], b_hh[2 * H:]])
    brow = brow1[None, :].astype(bf)
    wgen = np.ascontiguousarray(np.asarray(W_gen, np.float32).T).astype(bf)
    bgen = np.asarray(b_gen, np.float32)[None, :].astype(bf)

    in_maps = []
    for c in range(NCORES):
        fsh = feats[:, c * B:(c + 1) * B, :]                     # [T, B, C]
        featsT = np.ascontiguousarray(fsh.transpose(2, 1, 0)).astype(bf)
        featsS = np.ascontiguousarray(fsh.transpose(1, 0, 2)).astype(bf)
        in_maps.append({
            "featsT": featsT, "featsS": featsS, "wi2h": wi2h, "wh2h": wh2h,
            "bh2hr": bh2hr, "wcol": wcol, "wih": wih, "whh": whh,
            "brow": brow, "wgen": wgen, "bgen": bgen,
        })

    return in_maps


def kernel(**inputs):
    nc = _get_graph()
    in_maps = make_in_maps(**inputs)
    res = run_bass_kernel_spmd(nc, in_maps, core_ids=list(range(NCORES)))
    return np.concatenate(
        [res.results[c]["out"].transpose(1, 0, 2).reshape(B * L, NCLS)
         for c in range(NCORES)],
        axis=0)


# revision 25
# speedup vs baseline: 175.0350x; 1.0086x over previous
"""Bass/Trainium2 kernel for the attention-decoder problem (v3.1).

Data-parallel over batch: 8 cores x 32 batch each. Per core, a 26-step
Bahdanau-attention + GRU scan over T=128 encoder positions, then a
generator GEMM [832, 512] @ [512, 6736].

Structure:
- 2-way software pipeline over half-batches (16+16): half B runs half a
  step behind half A, so B's attention z-phase (Act-engine tanh, the
  per-step floor) overlaps A's serial tail (softmax, context, GRU) and
  vice versa; the Act engine stays ~continuously busy.
- gates are computed TRANSPOSED: stationary = W 128x128 chunks, moving =
  ctx^T/h^T [128, 16]; preactivations land [3H | b] in PSUM, gate math
  runs fully partition-parallel, and h' lands directly in [h | b] -- no
  transposes anywhere. The W_hh half of the gate GEMM only needs
  h(k-1), so it is issued at the start of the previous covering phase,
  off the critical chain; the W_ih half follows the context.
- context is computed TRANSPOSED: per (b, c-chunk) one matmul with
  stationary = feats_b chunk [t, c] and moving = the UNNORMALIZED
  exp(e) column; 1/s is broadcast across partitions via a k=1 matmul
  (computed in parallel with the context matmuls) and folded into the
  context PSUM->SBUF evacuation multiply.
- e^T accumulates per z-chunk into 4 PSUM columns right after each
  tanh; partial sums run early so only the last chunk's dot + one add
  sit on the critical chain.
- sigmoid(x) = 0.5 tanh(x/2) + 0.5 is algebraically folded so no gate
  fix-up op sits on the chain: with A' = 2*(W_ih,n ctx + b_ih,n)
  (weights pre-doubled on the host), n = tanh(0.5*(A' + (t_r+1)*gh_n))
  and h' = n + 0.5*(t_z+1)*(h-n), via scalar_tensor_tensor.
- all biases ride into PSUM as k=1 matmuls with stationary = bias-row.
- generator PSUM->SBUF copies run on the Act engine, queued exactly in
  the inter-phase boundary gap (while DVE runs the next z-add).
"""

import numpy as np
import ml_dtypes

import concourse.bass as bass
import concourse.mybir as mybir
import concourse.tile as tile
from concourse import bacc
from concourse.bass_utils import run_bass_kernel_spmd

BF16 = mybir.dt.bfloat16
F32 = mybir.dt.float32
AF = mybir.ActivationFunctionType
ALU = mybir.AluOpType

T, BFULL, C = 128, 256, 512
H, L, NCLS = 512, 26, 6736
NCORES = 8
B = BFULL // NCORES          # 32 per core
BH = B // 2                  # 16 per pipeline half
HC = H // 128                # 4 h-chunks
CC = C // 128                # 4 c-chunks
G3 = 3 * H                   # 1536
NSUB = 512                   # class-chunk width (one PSUM bank of f32)
NSUBS = [(i * NSUB, min(NSUB, NCLS - i * NSUB))
         for i in range((NCLS + NSUB - 1) // NSUB)]     # 14 chunks


def _gen_schedule():
    """sched[k] = list of (k0, nk, off, width) generator sub-GEMMs emitted
    during scan step k (group g: steps [4g, 4g+4), emitted over steps
    4g+4..4g+7, or 24..25 for g=5; g=6 runs after the scan)."""
    sched = {k: [] for k in range(L)}
    for g in range(6):
        k0, nk = 4 * g, 4
        if g < 5:
            # first step's chunks run in the B phase only (the other half's
            # h for the group's last step lands mid-A-phase); with the
            # 16-deep hist ring this is safe and keeps every boundary fed
            steps = [4 * g + 4] * 3 + [4 * g + 5] * 4 + [4 * g + 6] * 4 \
                + [4 * g + 7] * 3
        else:
            steps = [24] * 7 + [25] * 7
        for i, (off, width) in enumerate(NSUBS):
            sched[steps[i]].append((k0, nk, off, width))
    return sched

GEN_SCHED = _gen_schedule()
GEN_TAIL = [(24, 2, off, width) for (off, width) in NSUBS]


def build_graph(reps=1, skip=()):
    nc = bacc.Bacc(None, target_bir_lowering=False, debug=False)

    # ---- DRAM parameters (per-core shard shapes) ----
    featsT = nc.declare_dram_parameter("featsT", [C, B, T], BF16, isOutput=False)
    featsS = nc.declare_dram_parameter("featsS", [B, T, C], BF16, isOutput=False)
    wi2h = nc.declare_dram_parameter("wi2h", [C, H], BF16, isOutput=False)
    wh2h = nc.declare_dram_parameter("wh2h", [H, H], BF16, isOutput=False)
    bh2hr = nc.declare_dram_parameter("bh2hr", [1, H], BF16, isOutput=False)
    wcol = nc.declare_dram_parameter("wcol", [128, HC], BF16, isOutput=False)
    wih = nc.declare_dram_parameter("wih", [C, G3], BF16, isOutput=False)
    whh = nc.declare_dram_parameter("whh", [H, G3], BF16, isOutput=False)
    brow = nc.declare_dram_parameter("brow", [1, 4 * H], BF16, isOutput=False)
    wgen = nc.declare_dram_parameter("wgen", [H, NCLS], BF16, isOutput=False)
    bgen = nc.declare_dram_parameter("bgen", [1, NCLS], BF16, isOutput=False)
    out = nc.declare_dram_parameter("out", [L, B, NCLS], F32, isOutput=True)

    with tile.TileContext(nc) as tc:
        with tc.tile_pool(name="pp", bufs=1) as pp:
            # ---- persistent SBUF ----
            P_sb = pp.tile([128, HC, T, B], BF16)      # i2h proj, [h | t, b]
            fs_sb = pp.tile([128, B, C], BF16)         # feats [t | b, c]
            wh2h_sb = pp.tile([128, HC, H], BF16)
            bh2hr_sb = pp.tile([1, H], BF16)
            wcol_sb = pp.tile([128, HC], BF16)
            wih_sb = pp.tile([128, CC, G3], BF16)
            whh_sb = pp.tile([128, HC, G3], BF16)
            brow_sb = pp.tile([1, 4 * H], BF16)
            wgen_sb = pp.tile([128, HC, NCLS], BF16)
            # h^T ring buffer, slot-major: slot k%16 holds h_k^T [h | b].
            # 16-deep so generator reads of old slots never make the h'
            # write wait (WAR hazard with a lagging gen group).
            hist = pp.tile([128, HC, 16, B], BF16)
            hp_sb = pp.tile([128, HC, B], BF16)        # hp^T evac
            ctxT_sb = pp.tile([128, CC, B], BF16)      # ctx^T evac
            et_sb = pp.tile([128, B], F32)             # e^T running sum
            E_sb = pp.tile([128, B], BF16)             # exp(e^T), unnormalized
            recip_sb = pp.tile([1, B], BF16)           # 1/s
            bc_sb = pp.tile([128, 1, B], F32)          # 1/s bcast across t
            g2_sb = pp.tile([128, 8, B], F32)          # rz pre-act (hh+ih)
            rz_sb = pp.tile([128, 8, B], BF16)         # raw tanh(0.5 rz)
            n_sb = pp.tile([128, HC, B], F32)          # n gate
            t1_sb = pp.tile([128, HC, B], F32)         # gate temp
            ones128_sb = pp.tile([128, 1], BF16)
            ones1b_sb = pp.tile([1, 128], BF16)
            bgen_sb = pp.tile([1, NCLS], BF16)

            # ---- constant / weight loads, spread over 4 DMA rings so
            #      descriptor issue (~0.6us per dma_start per ring) overlaps ----
            nc.vector.memset(ones128_sb[:, :], 1.0)
            nc.vector.memset(ones1b_sb[:, :], 1.0)
            nc.vector.memset(hist[:, :, 15, :], 0.0)  # h_{-1} = 0 at slot 15
            for kc in range(HC):   # first use: hp at scan start
                nc.gpsimd.dma_start(wh2h_sb[:, kc, :],
                                    wh2h[kc * 128:(kc + 1) * 128, :])
            nc.gpsimd.dma_start(bh2hr_sb[:, :], bh2hr[:, :])
            nc.gpsimd.dma_start(wcol_sb[:, :], wcol[:, :])
            nc.gpsimd.dma_start(brow_sb[:, :], brow[:, :])
            nc.gpsimd.dma_start(bgen_sb[:, :], bgen[:, :])
            # big streams are scheduler-gated so their data transfers don't
            # contend with the prologue GEMM's featsT feed
            with tc.tile_wait_until(0.014):
                for b0 in range(0, B, 4):   # first use: ctx at ~60us
                    nc.gpsimd.dma_start(
                        fs_sb[:, b0:b0 + 4, :],
                        featsS[b0:b0 + 4, :, :].rearrange("b t c -> t b c"))
            with tc.tile_wait_until(0.030):  # first use ~58us (gates-hh k0)
                for kc in range(HC):
                    nc.gpsimd.dma_start(whh_sb[:, kc, :],
                                        whh[kc * 128:(kc + 1) * 128, :])
                for kc in range(CC):
                    nc.gpsimd.dma_start(wih_sb[:, kc, :],
                                        wih[kc * 128:(kc + 1) * 128, :])
            with tc.tile_wait_until(0.045):  # first use ~140us (gen step 4)
                for kc in range(HC):
                    nc.gpsimd.dma_start(wgen_sb[:, kc, :],
                                        wgen[kc * 128:(kc + 1) * 128, :])

            # ---- prologue: P^T = W_i2h^T @ feats^T, laid out [h | t, b] ----
            with (
                tc.tile_pool(name="ftp", bufs=2) as fp,
                tc.tile_pool(name="psP", bufs=4, space="PSUM") as psP,
            ):
                wi2h_sb = fp.tile([128, CC, H], BF16, tag="w", bufs=1)
                for kc in range(CC):
                    nc.scalar.dma_start(wi2h_sb[:, kc, :],
                                        wi2h[kc * 128:(kc + 1) * 128, :])
                # iterate over 4-sample groups (b-major) so half A's P is
                # complete at the halfway point and the scan starts early
                for bs in range(8):
                    ft_bs = fp.tile([128, CC, 4, T], BF16, tag="ft", bufs=5)
                    for kc in range(CC):
                        ring = nc.sync if kc % 2 == 0 else nc.scalar
                        ring.dma_start(
                            ft_bs[:, kc, :, :],
                            featsT[kc * 128:(kc + 1) * 128,
                                   bs * 4:(bs + 1) * 4, :])
                    for mp in range(0, HC, 2):
                        pps = [psP.tile([128, 512], F32, tag="pp",
                                        name=f"ppsum{q}") for q in (mp, mp + 1)]
                        for kc in range(CC):
                            for mc, ppsum in zip((mp, mp + 1), pps):
                                nc.tensor.matmul(
                                    ppsum[:, :],
                                    wi2h_sb[:, kc, mc * 128:(mc + 1) * 128],
                                    ft_bs[:, kc, :, :],
                                    start=(kc == 0), stop=(kc == CC - 1))
                        for mc, ppsum in zip((mp, mp + 1), pps):
                            dst = P_sb[:, mc, :, bs * 4:(bs + 1) * 4]
                            src_ap = ppsum[:, :].rearrange(
                                "p (b t) -> p t b", b=4)
                            if mc % 2 == 0:
                                nc.vector.tensor_copy(dst, src_ap)
                            else:
                                nc.scalar.activation(dst, src_ap, AF.Copy)

            state = [None, None]   # per-half carry between phases
            genct = [0]            # alternate gen-copy engine

            def emit_hp(h0, k):
                """hp^T = W_h2h^T h^T + b -> PSUM; evac chunk 0 first so the
                first z-add can start before the rest lands."""
                hp_ps = psS.tile([128, HC, BH], F32, tag="sm")
                for mc in range(HC):
                    for kc in range(HC):
                        nc.tensor.matmul(
                            hp_ps[:, mc, :],
                            wh2h_sb[:, kc, mc * 128:(mc + 1) * 128],
                            hist[:, kc, (k - 1) % 16, h0:h0 + BH],
                            start=(kc == 0), stop=False)
                    nc.tensor.matmul(
                        hp_ps[:, mc, :],
                        bh2hr_sb[0:1, mc * 128:(mc + 1) * 128],
                        ones1b_sb[0:1, 0:BH], start=False, stop=True)
                nc.vector.tensor_copy(hp_sb[:, 0, h0:h0 + BH], hp_ps[:, 0, :])
                nc.vector.tensor_copy(hp_sb[:, 1:, h0:h0 + BH], hp_ps[:, 1:, :])

            def emit_gates_hh(h0, k):
                """W_hh part of the gate GEMM + n_h bias; only needs h(k-1),
                so it runs at the covering phase's start, off-chain.
                g tile regions: 0-7 rz(hh), 8-11 n_h(+b_hh,n),
                12-19 rz(ih)+b_rz, 20-23 2*(n_i+b_ih,n)."""
                g_ps = psG.tile([128, 24, BH], F32, tag="g")
                state[h0 // BH]["g"] = g_ps
                if "gates" in skip:
                    return
                hT = hist[:, :, (k - 1) % 16, :]
                for mi in range(12):
                    col = (mi % 4) * 128 + (0 if mi < 4 else
                                            512 if mi < 8 else 1024)
                    for kc in range(HC):
                        nc.tensor.matmul(
                            g_ps[:, mi, :], whh_sb[:, kc, col:col + 128],
                            hT[:, kc, h0:h0 + BH],
                            start=(kc == 0), stop=(mi < 8 and kc == HC - 1))
                    if mi >= 8:   # + b_hh,n
                        boff = 1536 + (mi - 8) * 128
                        nc.tensor.matmul(
                            g_ps[:, mi, :], brow_sb[0:1, boff:boff + 128],
                            ones1b_sb[0:1, 0:BH], start=False, stop=True)

            def emit_z(h0, hc, split=False):
                z = zp.tile([128, T, BH], BF16, tag="z")
                spans = ((0, T // 2), (T // 2, T)) if split else ((0, T),)
                for (t0, t1) in spans:
                    nc.vector.tensor_tensor(
                        z[:, t0:t1, :], P_sb[:, hc, t0:t1, h0:h0 + BH],
                        hp_sb[:, hc:hc + 1, h0:h0 + BH]
                        .broadcast_to((128, t1 - t0, BH)),
                        op=ALU.add)
                    if "tanh" not in skip:
                        nc.scalar.activation(z[:, t0:t1, :], z[:, t0:t1, :],
                                             AF.Tanh)
                return z

            def emit_e(z, hc, e_ps, h0, half=None):
                """one e^T column set from z chunk hc + running et update.
                half=0/1 restricts to t rows [0,64)/[64,128) (chunk-3 split
                so the first half overlaps the second half's tanh)."""
                if "e" in skip:
                    if hc == 0:
                        nc.vector.memset(et_sb[:, h0:h0 + BH], 0.5)
                    return
                r0, r1 = (0, T) if half is None else (half * 64, half * 64 + 64)
                for b in range(BH):
                    nc.tensor.matmul(
                        e_ps[r0:r1, hc, b:b + 1], z[:, r0:r1, b],
                        wcol_sb[:, hc:hc + 1], start=True, stop=True)
                if hc == 0:
                    nc.vector.tensor_copy(et_sb[:, h0:h0 + BH], e_ps[:, 0, :])
                elif hc == 1:
                    nc.vector.tensor_tensor(et_sb[:, h0:h0 + BH],
                                            et_sb[:, h0:h0 + BH],
                                            e_ps[:, 1, :], op=ALU.add)
                elif hc == 2:
                    nc.vector.tensor_tensor(et_sb[:, h0:h0 + BH],
                                            et_sb[:, h0:h0 + BH],
                                            e_ps[:, 2, :], op=ALU.add)
                elif hc == 3:
                    nc.vector.tensor_tensor(et_sb[r0:r1, h0:h0 + BH],
                                            et_sb[r0:r1, h0:h0 + BH],
                                            e_ps[r0:r1, 3, :], op=ALU.add)

            def emit_gen_mm(k0, nk, off, width):
                m = B * nk
                s0 = k0 % 16
                o_ps = psO.tile([128, NSUB], F32, tag="gen")
                for hc in range(HC):
                    nc.tensor.matmul(
                        o_ps[:m, 0:width],
                        hist[:, hc, s0:s0 + nk, :],
                        wgen_sb[:, hc, off:off + width],
                        start=(hc == 0), stop=False)
                nc.tensor.matmul(
                    o_ps[:m, 0:width], ones1b_sb[0:1, 0:m],
                    bgen_sb[0:1, off:off + width], start=False, stop=True)
                return o_ps, m

            def emit_gen_out(o_ps, m, k0, nk, off, width, on_act=True,
                             ring=None):
                o_sb = gop.tile([128, NSUB], F32, tag="ob")
                if on_act:
                    nc.scalar.activation(o_sb[:m, 0:width], o_ps[:m, 0:width],
                                         AF.Copy)
                else:
                    nc.vector.tensor_copy(o_sb[:m, 0:width], o_ps[:m, 0:width])
                if ring is None:
                    genct[0] += 1
                    ring = nc.sync if genct[0] % 2 == 0 else nc.scalar
                ring.dma_start(
                    out[k0:k0 + nk, :, off:off + width], o_sb[:m, 0:width])

            # ---- tail sub-blocks for half at h0 (its z-phase ran last
            #      phase; e_ps columns 0-2 are already summed in et_sb) ----
            def tail_softmax(h0):
                st = state[h0 // BH]
                nc.scalar.activation(E_sb[:, h0:h0 + BH], et_sb[:, h0:h0 + BH],
                                     AF.Exp)
                s_ps = psS.tile([1, BH], F32, tag="sm")
                nc.tensor.matmul(s_ps[:, :], ones128_sb[:, :],
                                 E_sb[:, h0:h0 + BH], start=True, stop=True)
                with nc.allow_low_precision(reason="1/s bf16; ctx is bf16"):
                    nc.vector.reciprocal(recip_sb[:, h0:h0 + BH], s_ps[:, :])
                bc_ps = psS.tile([128, 1, BH], F32, tag="sm")
                nc.tensor.matmul(bc_ps[:, 0, :], ones1b_sb[0:1, :],
                                 recip_sb[:, h0:h0 + BH], start=True, stop=True)
                nc.vector.tensor_copy(bc_sb[:, :, h0:h0 + BH], bc_ps[:, :, :])

            def tail_ctx_mm(h0):
                st = state[h0 // BH]
                if "ctx" in skip:
                    return
                ctx_ps = psS.tile([128, CC, BH], F32, tag="sm")
                st["ctx"] = ctx_ps
                for b in range(BH):
                    for cc in range(CC):
                        nc.tensor.matmul(
                            ctx_ps[:, cc, b:b + 1],
                            fs_sb[:, h0 + b, cc * 128:(cc + 1) * 128],
                            E_sb[:, h0 + b:h0 + b + 1],
                            start=True, stop=True)

            def tail_ctx_evac(h0):
                if "ctx" in skip:
                    nc.vector.memset(ctxT_sb[:, :, h0:h0 + BH], 0.01)
                    return
                ctx_ps = state[h0 // BH]["ctx"]
                # evac with the 1/s scale folded in
                nc.vector.tensor_tensor(
                    ctxT_sb[:, :, h0:h0 + BH], ctx_ps[:, :, :],
                    bc_sb[:, :, h0:h0 + BH].broadcast_to((128, CC, BH)),
                    op=ALU.mult)

            def tail_gates_ih(h0):
                if "gates" in skip:
                    return
                g_ps = state[h0 // BH]["g"]
                for mi in range(12, 24):
                    col = ((mi - 12) % 4) * 128 + (0 if mi < 16 else
                                                   512 if mi < 20 else 1024)
                    for kc in range(CC):
                        nc.tensor.matmul(
                            g_ps[:, mi, :], wih_sb[:, kc, col:col + 128],
                            ctxT_sb[:, kc, h0:h0 + BH],
                            start=(kc == 0), stop=False)
                    boff = (mi - 12) * 128 if mi < 20 else 1024 + (mi - 20) * 128
                    nc.tensor.matmul(
                        g_ps[:, mi, :], brow_sb[0:1, boff:boff + 128],
                        ones1b_sb[0:1, 0:BH], start=False, stop=True)
                # rz preactivation = hh part + ih part (the hh evac runs
                # early, off-chain: its matmuls finished at phase start)
                nc.vector.tensor_copy(g2_sb[:, :, h0:h0 + BH], g_ps[:, 0:8, :])
                nc.vector.tensor_tensor(g2_sb[:, :, h0:h0 + BH],
                                        g2_sb[:, :, h0:h0 + BH],
                                        g_ps[:, 12:20, :], op=ALU.add)
                nc.scalar.activation(rz_sb[:, :, h0:h0 + BH],
                                     g2_sb[:, :, h0:h0 + BH], AF.Tanh,
                                     scale=0.5)

            def tail_n_h(h0, k):
                if "gates" in skip:
                    nc.vector.memset(hist[:, :, k % 16, h0:h0 + BH], 0.01)
                    return
                g_ps = state[h0 // BH]["g"]
                hprev = hist[:, :, (k - 1) % 16, h0:h0 + BH]
                t1 = t1_sb[:, :, h0:h0 + BH]
                n = n_sb[:, :, h0:h0 + BH]
                # n = tanh(0.5*(A' + (t_r+1)*gh_n)), A' = 2*(W_ih,n ctx + b)
                nc.vector.scalar_tensor_tensor(
                    t1, rz_sb[:, 0:4, h0:h0 + BH], 1.0, g_ps[:, 8:12, :],
                    ALU.add, ALU.mult)
                nc.vector.tensor_tensor(t1, t1, g_ps[:, 20:24, :], op=ALU.add)
                nc.scalar.activation(n, t1, AF.Tanh, scale=0.5)
                # h' = n + 0.5*(t_z+1)*(h-n)
                nc.vector.tensor_tensor(t1, hprev, n, op=ALU.subtract)
                nc.vector.scalar_tensor_tensor(
                    t1, rz_sb[:, 4:8, h0:h0 + BH], 1.0, t1, ALU.add, ALU.mult)
                nc.vector.scalar_tensor_tensor(
                    hist[:, :, k % 16, h0:h0 + BH], t1, 0.5, n,
                    ALU.mult, ALU.add)

            def emit_phase(h0, k, tail_h0, tail_k, gen_items):
                """z-phase for half h0 at step k, with the other half's tail
                (for step tail_k) interleaved at fixed points."""
                me = state[h0 // BH] = {}
                do_tail = tail_k >= 0 and "tail" not in skip
                emit_hp(h0, k)
                emit_gates_hh(h0, k)
                e_ps = psS.tile([128, HC, BH], F32, tag="e", bufs=1)
                me["e"] = e_ps
                if do_tail:
                    # softmax + ctx matmuls first: the exp->1/s->ctx->gates
                    # chain is ~4.5us and must finish before rz (after z2)
                    tail_softmax(tail_h0)
                    tail_ctx_mm(tail_h0)
                zts = [emit_z(h0, 0, split=True)]
                zts.append(emit_z(h0, 1))
                if do_tail:
                    tail_ctx_evac(tail_h0)
                zts.append(emit_z(h0, 2))
                if do_tail:
                    tail_gates_ih(tail_h0)
                emit_e(zts[0], 0, e_ps, h0)
                emit_e(zts[1], 1, e_ps, h0)
                if do_tail:
                    tail_n_h(tail_h0, tail_k)
                zts.append(emit_z(h0, 3, split=True))
                emit_e(zts[2], 2, e_ps, h0)
                gen_work = [emit_gen_mm(*gi) + gi for gi in gen_items]
                emit_e(zts[3], 3, e_ps, h0, half=0)
                emit_e(zts[3], 3, e_ps, h0, half=1)
                # gen evacuations land in the boundary gap while DVE runs
                # the next phase's first z-add
                for (o_ps, m, k0, nk, off, width) in gen_work:
                    emit_gen_out(o_ps, m, k0, nk, off, width)

            def emit_tail(h0, k):
                tail_softmax(h0)
                tail_ctx_mm(h0)
                tail_ctx_evac(h0)
                tail_gates_ih(h0)
                tail_n_h(h0, k)

            from contextlib import ExitStack
            for rep in range(reps):
                if rep > 0:
                    nc.vector.memset(hist[:, :, 15, :], 0.0)
                state[0] = {}
                state[1] = {}
                with ExitStack() as scan_pools:
                    zp = scan_pools.enter_context(
                        tc.tile_pool(name=f"zp{rep}", bufs=6))
                    gop = scan_pools.enter_context(
                        tc.tile_pool(name=f"gop{rep}", bufs=3))
                    psS = scan_pools.enter_context(
                        tc.tile_pool(name=f"psS{rep}", bufs=2, space="PSUM"))
                    psG = scan_pools.enter_context(
                        tc.tile_pool(name=f"psG{rep}", bufs=2, space="PSUM"))
                    psO = scan_pools.enter_context(
                        tc.tile_pool(name=f"psO{rep}", bufs=3, space="PSUM"))
                    for k in range(L):
                        gi = [] if "gen" in skip else GEN_SCHED[k]
                        # a group's first emission step: the B half's h for
                        # the group's last step lands mid-A-phase; emitting
                        # its chunks in the A phase would head-of-line block
                        # PE behind that write, so they all go to the B phase
                        if any(k0 + nk == k for (k0, nk, _, _) in gi):
                            ga, gb = [], gi
                        else:
                            ga, gb = gi[:len(gi) // 2], gi[len(gi) // 2:]
                        emit_phase(0, k, BH, k - 1, ga)
                        emit_phase(BH, k, 0, k, gb)
                    if "tail" not in skip:
                        emit_tail(BH, L - 1)

                # ---- generator tail: rows for steps 24..25, on wide pools
                #      so the 14 chunks pipeline instead of serializing ----
                with (
                    tc.tile_pool(name=f"gopD{rep}", bufs=6) as gopD,
                    tc.tile_pool(name=f"psD{rep}", bufs=6, space="PSUM") as psD,
                ):
                    gop, psO = gopD, psD
                    for i, (k0, nk, off, width) in enumerate(
                            [] if "gen" in skip else GEN_TAIL):
                        o_ps, m = emit_gen_mm(k0, nk, off, width)
                        emit_gen_out(o_ps, m, k0, nk, off, width,
                                     on_act=(i % 2 == 0),
                                     ring=(nc.sync if i % 2 == 0 else
                                           nc.scalar))

    nc.finalize()
    return nc


def _get_graph():
    if not hasattr(_get_graph, "_nc"):
        _get_graph._nc = build_graph()
    return _get_graph._nc


def make_in_maps(feats, text_length, W_i2h, W_h2h, b_h2h, W_score, W_ih, W_hh,
                 b_ih, b_hh, W_gen, b_gen):
    bf = ml_dtypes.bfloat16
    feats = np.asarray(feats, np.float32)

    wi2h = np.ascontiguousarray(np.asarray(W_i2h, np.float32).T).astype(bf)
    wh2h = np.ascontiguousarray(np.asarray(W_h2h, np.float32).T).astype(bf)
    bh2hr = np.asarray(b_h2h, np.float32)[None, :].astype(bf)
    wcol = np.ascontiguousarray(
        np.asarray(W_score, np.float32)[0].reshape(HC, 128).T).astype(bf)
    # n-gate input weights/bias doubled: n = tanh(0.5*(A' + (t_r+1)*gh_n))
    wih_f = np.ascontiguousarray(np.asarray(W_ih, np.float32).T).copy()
    wih_f[:, 2 * H:] *= 2.0
    wih = wih_f.astype(bf)
    whh = np.ascontiguousarray(np.asarray(W_hh, np.float32).T).astype(bf)
    b_ih = np.asarray(b_ih, np.float32)
    b_hh = np.asarray(b_hh, np.float32)
    brow1 = np.concatenate([b_ih[:2 * H] + b_hh[:2 * H],
                            2.0 * b_ih[2 * H:], b_hh[2 * H:]])
    brow = brow1[None, :].astype(bf)
    wgen = np.ascontiguousarray(np.asarray(W_gen, np.float32).T).astype(bf)
    bgen = np.asarray(b_gen, np.float32)[None, :].astype(bf)

    in_maps = []
    for c in range(NCORES):
        fsh = feats[:, c * B:(c + 1) * B, :]                     # [T, B, C]
        featsT = np.ascontiguousarray(fsh.transpose(2, 1, 0)).astype(bf)
        featsS = np.ascontiguousarray(fsh.transpose(1, 0, 2)).astype(bf)
        in_maps.append({
            "featsT": featsT, "featsS": featsS, "wi2h": wi2h, "wh2h": wh2h,
            "bh2hr": bh2hr, "wcol": wcol, "wih": wih, "whh": whh,
            "brow": brow, "wgen": wgen, "bgen": bgen,
        })

    return in_maps


def kernel(**inputs):
    nc = _get_graph()
    in_maps = make_in_maps(**inputs)
    res = run_bass_kernel_spmd(nc, in_maps, core_ids=list(range(NCORES)))
    return np.concatenate(
        [res.results[c]["out"].transpose(1, 0, 2).reshape(B * L, NCLS)
         for c in range(NCORES)],
        axis=0)


# revision 26
# speedup vs baseline: 177.0210x; 1.0113x over previous
"""Bass/Trainium2 kernel for the attention-decoder problem (v3.1).

Data-parallel over batch: 8 cores x 32 batch each. Per core, a 26-step
Bahdanau-attention + GRU scan over T=128 encoder positions, then a
generator GEMM [832, 512] @ [512, 6736].

Structure:
- 2-way software pipeline over half-batches (16+16): half B runs half a
  step behind half A, so B's attention z-phase (Act-engine tanh, the
  per-step floor) overlaps A's serial tail (softmax, context, GRU) and
  vice versa; the Act engine stays ~continuously busy.
- gates are computed TRANSPOSED: stationary = W 128x128 chunks, moving =
  ctx^T/h^T [128, 16]; preactivations land [3H | b] in PSUM, gate math
  runs fully partition-parallel, and h' lands directly in [h | b] -- no
  transposes anywhere. The W_hh half of the gate GEMM only needs
  h(k-1), so it is issued at the start of the previous covering phase,
  off the critical chain; the W_ih half follows the context.
- context is computed TRANSPOSED: per (b, c-chunk) one matmul with
  stationary = feats_b chunk [t, c] and moving = the UNNORMALIZED
  exp(e) column; 1/s is broadcast across partitions via a k=1 matmul
  (computed in parallel with the context matmuls) and folded into the
  context PSUM->SBUF evacuation multiply.
- e^T accumulates per z-chunk into 4 PSUM columns right after each
  tanh; partial sums run early so only the last chunk's dot + one add
  sit on the critical chain.
- sigmoid(x) = 0.5 tanh(x/2) + 0.5 is algebraically folded so no gate
  fix-up op sits on the chain: with A' = 2*(W_ih,n ctx + b_ih,n)
  (weights pre-doubled on the host), n = tanh(0.5*(A' + (t_r+1)*gh_n))
  and h' = n + 0.5*(t_z+1)*(h-n), via scalar_tensor_tensor.
- all biases ride into PSUM as k=1 matmuls with stationary = bias-row.
- generator PSUM->SBUF copies run on the Act engine, queued exactly in
  the inter-phase boundary gap (while DVE runs the next z-add).
"""

import numpy as np
import ml_dtypes

import concourse.bass as bass
import concourse.mybir as mybir
import concourse.tile as tile
from concourse import bacc
from concourse.bass_utils import run_bass_kernel_spmd

BF16 = mybir.dt.bfloat16
F32 = mybir.dt.float32
AF = mybir.ActivationFunctionType
ALU = mybir.AluOpType

T, BFULL, C = 128, 256, 512
H, L, NCLS = 512, 26, 6736
NCORES = 8
B = BFULL // NCORES          # 32 per core
BH = B // 2                  # 16 per pipeline half
HC = H // 128                # 4 h-chunks
CC = C // 128                # 4 c-chunks
G3 = 3 * H                   # 1536
NSUB = 512                   # class-chunk width (one PSUM bank of f32)
NSUBS = [(i * NSUB, min(NSUB, NCLS - i * NSUB))
         for i in range((NCLS + NSUB - 1) // NSUB)]     # 14 chunks


def _gen_schedule():
    """sched[k] = list of (k0, nk, off, width) generator sub-GEMMs emitted
    during scan step k (group g: steps [4g, 4g+4), emitted over steps
    4g+4..4g+7, or 24..25 for g=5; g=6 runs after the scan)."""
    sched = {k: [] for k in range(L)}
    for g in range(6):
        k0, nk = 4 * g, 4
        if g < 5:
            # first step's chunks run in the B phase only (the other half's
            # h for the group's last step lands mid-A-phase); with the
            # 16-deep hist ring this is safe and keeps every boundary fed
            steps = [4 * g + 4] * 3 + [4 * g + 5] * 4 + [4 * g + 6] * 4 \
                + [4 * g + 7] * 3
        else:
            steps = [24] * 7 + [25] * 7
        for i, (off, width) in enumerate(NSUBS):
            sched[steps[i]].append((k0, nk, off, width))
    return sched

GEN_SCHED = _gen_schedule()
GEN_TAIL = [(24, 2, off, width) for (off, width) in NSUBS]


def build_graph(reps=1, skip=()):
    nc = bacc.Bacc(None, target_bir_lowering=False, debug=False)

    # ---- DRAM parameters (per-core shard shapes) ----
    featsT = nc.declare_dram_parameter("featsT", [C, B, T], BF16, isOutput=False)
    featsS = nc.declare_dram_parameter("featsS", [B, T, C], BF16, isOutput=False)
    wi2h = nc.declare_dram_parameter("wi2h", [C, H], BF16, isOutput=False)
    wh2h = nc.declare_dram_parameter("wh2h", [H, H], BF16, isOutput=False)
    bh2hr = nc.declare_dram_parameter("bh2hr", [1, H], BF16, isOutput=False)
    wcol = nc.declare_dram_parameter("wcol", [128, HC], BF16, isOutput=False)
    wih = nc.declare_dram_parameter("wih", [C, G3], BF16, isOutput=False)
    whh = nc.declare_dram_parameter("whh", [H, G3], BF16, isOutput=False)
    brow = nc.declare_dram_parameter("brow", [1, 4 * H], BF16, isOutput=False)
    wgen = nc.declare_dram_parameter("wgen", [H, NCLS], BF16, isOutput=False)
    bgen = nc.declare_dram_parameter("bgen", [1, NCLS], BF16, isOutput=False)
    out = nc.declare_dram_parameter("out", [L, B, NCLS], F32, isOutput=True)

    with tile.TileContext(nc) as tc:
        with tc.tile_pool(name="pp", bufs=1) as pp:
            # ---- persistent SBUF ----
            P_sb = pp.tile([128, HC, T, B], BF16)      # i2h proj, [h | t, b]
            fs_sb = pp.tile([128, B, C], BF16)         # feats [t | b, c]
            wh2h_sb = pp.tile([128, HC, H], BF16)
            bh2hr_sb = pp.tile([1, H], BF16)
            wcol_sb = pp.tile([128, HC], BF16)
            wih_sb = pp.tile([128, CC, G3], BF16)
            whh_sb = pp.tile([128, HC, G3], BF16)
            brow_sb = pp.tile([1, 4 * H], BF16)
            wgen_sb = pp.tile([128, HC, NCLS], BF16)
            # h^T ring buffer, slot-major: slot k%16 holds h_k^T [h | b].
            # 16-deep so generator reads of old slots never make the h'
            # write wait (WAR hazard with a lagging gen group).
            hist = pp.tile([128, HC, 16, B], BF16)
            hp_sb = pp.tile([128, HC, B], BF16)        # hp^T evac
            ctxT_sb = pp.tile([128, CC, B], BF16)      # ctx^T evac
            et_sb = pp.tile([128, B], F32)             # e^T running sum
            E_sb = pp.tile([128, B], BF16)             # exp(e^T), unnormalized
            recip_sb = pp.tile([1, B], BF16)           # 1/s
            bc_sb = pp.tile([128, 1, B], F32)          # 1/s bcast across t
            g2_sb = pp.tile([128, 8, B], F32)          # rz pre-act (hh+ih)
            rz_sb = pp.tile([128, 8, B], BF16)         # raw tanh(0.5 rz)
            n_sb = pp.tile([128, HC, B], F32)          # n gate
            t1_sb = pp.tile([128, HC, B], F32)         # gate temp
            wi2h_pre = pp.tile([128, CC, H], BF16)
            ones128_sb = pp.tile([128, 1], BF16)
            ones1b_sb = pp.tile([1, 128], BF16)
            bgen_sb = pp.tile([1, NCLS], BF16)

            # ---- constant / weight loads, spread over 4 DMA rings so
            #      descriptor issue (~0.6us per dma_start per ring) overlaps ----
            nc.vector.memset(ones128_sb[:, :], 1.0)
            nc.vector.memset(ones1b_sb[:, :], 1.0)
            nc.vector.memset(hist[:, :, 15, :], 0.0)  # h_{-1} = 0 at slot 15
            for kc in range(CC):   # stationary for the prologue GEMM
                nc.gpsimd.dma_start(wi2h_pre[:, kc, :],
                                    wi2h[kc * 128:(kc + 1) * 128, :])
            for kc in range(HC):   # first use: hp at scan start
                nc.gpsimd.dma_start(wh2h_sb[:, kc, :],
                                    wh2h[kc * 128:(kc + 1) * 128, :])
            nc.gpsimd.dma_start(bh2hr_sb[:, :], bh2hr[:, :])
            nc.gpsimd.dma_start(wcol_sb[:, :], wcol[:, :])
            nc.gpsimd.dma_start(brow_sb[:, :], brow[:, :])
            nc.gpsimd.dma_start(bgen_sb[:, :], bgen[:, :])
            # big streams are scheduler-gated so their data transfers don't
            # contend with the prologue GEMM's featsT feed
            with tc.tile_wait_until(0.014):
                for b0 in range(0, B, 4):   # first use: ctx at ~60us
                    nc.gpsimd.dma_start(
                        fs_sb[:, b0:b0 + 4, :],
                        featsS[b0:b0 + 4, :, :].rearrange("b t c -> t b c"))
            with tc.tile_wait_until(0.030):  # first use ~58us (gates-hh k0)
                for kc in range(HC):
                    nc.gpsimd.dma_start(whh_sb[:, kc, :],
                                        whh[kc * 128:(kc + 1) * 128, :])
                for kc in range(CC):
                    nc.gpsimd.dma_start(wih_sb[:, kc, :],
                                        wih[kc * 128:(kc + 1) * 128, :])
            with tc.tile_wait_until(0.045):  # first use ~140us (gen step 4)
                for kc in range(HC):
                    nc.gpsimd.dma_start(wgen_sb[:, kc, :],
                                        wgen[kc * 128:(kc + 1) * 128, :])

            # ---- prologue: P^T = W_i2h^T @ feats^T, laid out [h | t, b] ----
            with (
                tc.tile_pool(name="ftp", bufs=2) as fp,
                tc.tile_pool(name="psP", bufs=4, space="PSUM") as psP,
            ):
                wi2h_sb = wi2h_pre
                # iterate over 4-sample groups (b-major) so half A's P is
                # complete at the halfway point and the scan starts early
                rings = (nc.sync, nc.scalar, nc.sync, nc.scalar)
                for bs in range(8):
                    ft_bs = fp.tile([128, CC, 4, T], BF16, tag="ft", bufs=5)
                    for kc in range(CC):
                        rings[kc].dma_start(
                            ft_bs[:, kc, :, :],
                            featsT[kc * 128:(kc + 1) * 128,
                                   bs * 4:(bs + 1) * 4, :])
                    for mp in range(0, HC, 2):
                        pps = [psP.tile([128, 512], F32, tag="pp",
                                        name=f"ppsum{q}") for q in (mp, mp + 1)]
                        for kc in range(CC):
                            for mc, ppsum in zip((mp, mp + 1), pps):
                                nc.tensor.matmul(
                                    ppsum[:, :],
                                    wi2h_sb[:, kc, mc * 128:(mc + 1) * 128],
                                    ft_bs[:, kc, :, :],
                                    start=(kc == 0), stop=(kc == CC - 1))
                        for mc, ppsum in zip((mp, mp + 1), pps):
                            dst = P_sb[:, mc, :, bs * 4:(bs + 1) * 4]
                            src_ap = ppsum[:, :].rearrange(
                                "p (b t) -> p t b", b=4)
                            if mc % 2 == 0:
                                nc.vector.tensor_copy(dst, src_ap)
                            else:
                                nc.scalar.activation(dst, src_ap, AF.Copy)

            state = [None, None]   # per-half carry between phases
            genct = [0]            # alternate gen-copy engine

            def emit_hp(h0, k):
                """hp^T = W_h2h^T h^T + b -> PSUM; evac chunk 0 first so the
                first z-add can start before the rest lands."""
                hp_ps = psS.tile([128, HC, BH], F32, tag="sm")
                for mc in range(HC):
                    for kc in range(HC):
                        nc.tensor.matmul(
                            hp_ps[:, mc, :],
                            wh2h_sb[:, kc, mc * 128:(mc + 1) * 128],
                            hist[:, kc, (k - 1) % 16, h0:h0 + BH],
                            start=(kc == 0), stop=False)
                    nc.tensor.matmul(
                        hp_ps[:, mc, :],
                        bh2hr_sb[0:1, mc * 128:(mc + 1) * 128],
                        ones1b_sb[0:1, 0:BH], start=False, stop=True)
                nc.vector.tensor_copy(hp_sb[:, 0, h0:h0 + BH], hp_ps[:, 0, :])
                nc.vector.tensor_copy(hp_sb[:, 1:, h0:h0 + BH], hp_ps[:, 1:, :])

            def emit_gates_hh(h0, k):
                """W_hh part of the gate GEMM + n_h bias; only needs h(k-1),
                so it runs at the covering phase's start, off-chain.
                g tile regions: 0-7 rz(hh), 8-11 n_h(+b_hh,n),
                12-19 rz(ih)+b_rz, 20-23 2*(n_i+b_ih,n)."""
                g_ps = psG.tile([128, 24, BH], F32, tag="g")
                state[h0 // BH]["g"] = g_ps
                if "gates" in skip:
                    return
                hT = hist[:, :, (k - 1) % 16, :]
                for mi in range(12):
                    col = (mi % 4) * 128 + (0 if mi < 4 else
                                            512 if mi < 8 else 1024)
                    for kc in range(HC):
                        nc.tensor.matmul(
                            g_ps[:, mi, :], whh_sb[:, kc, col:col + 128],
                            hT[:, kc, h0:h0 + BH],
                            start=(kc == 0), stop=(mi < 8 and kc == HC - 1))
                    if mi >= 8:   # + b_hh,n
                        boff = 1536 + (mi - 8) * 128
                        nc.tensor.matmul(
                            g_ps[:, mi, :], brow_sb[0:1, boff:boff + 128],
                            ones1b_sb[0:1, 0:BH], start=False, stop=True)

            def emit_z(h0, hc, split=False):
                z = zp.tile([128, T, BH], BF16, tag="z")
                spans = ((0, T // 2), (T // 2, T)) if split else ((0, T),)
                for (t0, t1) in spans:
                    nc.vector.tensor_tensor(
                        z[:, t0:t1, :], P_sb[:, hc, t0:t1, h0:h0 + BH],
                        hp_sb[:, hc:hc + 1, h0:h0 + BH]
                        .broadcast_to((128, t1 - t0, BH)),
                        op=ALU.add)
                    if "tanh" not in skip:
                        nc.scalar.activation(z[:, t0:t1, :], z[:, t0:t1, :],
                                             AF.Tanh)
                return z

            def emit_e(z, hc, e_ps, h0, half=None):
                """one e^T column set from z chunk hc + running et update.
                half=0/1 restricts to t rows [0,64)/[64,128) (chunk-3 split
                so the first half overlaps the second half's tanh)."""
                if "e" in skip:
                    if hc == 0:
                        nc.vector.memset(et_sb[:, h0:h0 + BH], 0.5)
                    return
                r0, r1 = (0, T) if half is None else (half * 64, half * 64 + 64)
                for b in range(BH):
                    nc.tensor.matmul(
                        e_ps[r0:r1, hc, b:b + 1], z[:, r0:r1, b],
                        wcol_sb[:, hc:hc + 1], start=True, stop=True)
                if hc == 0:
                    nc.vector.tensor_copy(et_sb[:, h0:h0 + BH], e_ps[:, 0, :])
                elif hc == 1:
                    nc.vector.tensor_tensor(et_sb[:, h0:h0 + BH],
                                            et_sb[:, h0:h0 + BH],
                                            e_ps[:, 1, :], op=ALU.add)
                elif hc == 2:
                    nc.vector.tensor_tensor(et_sb[:, h0:h0 + BH],
                                            et_sb[:, h0:h0 + BH],
                                            e_ps[:, 2, :], op=ALU.add)
                elif hc == 3:
                    nc.vector.tensor_tensor(et_sb[r0:r1, h0:h0 + BH],
                                            et_sb[r0:r1, h0:h0 + BH],
                                            e_ps[r0:r1, 3, :], op=ALU.add)

            def emit_gen_mm(k0, nk, off, width):
                m = B * nk
                s0 = k0 % 16
                o_ps = psO.tile([128, NSUB], F32, tag="gen")
                for hc in range(HC):
                    nc.tensor.matmul(
                        o_ps[:m, 0:width],
                        hist[:, hc, s0:s0 + nk, :],
                        wgen_sb[:, hc, off:off + width],
                        start=(hc == 0), stop=False)
                nc.tensor.matmul(
                    o_ps[:m, 0:width], ones1b_sb[0:1, 0:m],
                    bgen_sb[0:1, off:off + width], start=False, stop=True)
                return o_ps, m

            def emit_gen_out(o_ps, m, k0, nk, off, width, on_act=True,
                             ring=None):
                o_sb = gop.tile([128, NSUB], F32, tag="ob")
                if on_act:
                    nc.scalar.activation(o_sb[:m, 0:width], o_ps[:m, 0:width],
                                         AF.Copy)
                else:
                    nc.vector.tensor_copy(o_sb[:m, 0:width], o_ps[:m, 0:width])
                if ring is None:
                    genct[0] += 1
                    ring = nc.sync if genct[0] % 2 == 0 else nc.scalar
                ring.dma_start(
                    out[k0:k0 + nk, :, off:off + width], o_sb[:m, 0:width])

            # ---- tail sub-blocks for half at h0 (its z-phase ran last
            #      phase; e_ps columns 0-2 are already summed in et_sb) ----
            def tail_softmax(h0):
                st = state[h0 // BH]
                nc.scalar.activation(E_sb[:, h0:h0 + BH], et_sb[:, h0:h0 + BH],
                                     AF.Exp)
                s_ps = psS.tile([1, BH], F32, tag="sm")
                nc.tensor.matmul(s_ps[:, :], ones128_sb[:, :],
                                 E_sb[:, h0:h0 + BH], start=True, stop=True)
                with nc.allow_low_precision(reason="1/s bf16; ctx is bf16"):
                    nc.vector.reciprocal(recip_sb[:, h0:h0 + BH], s_ps[:, :])
                bc_ps = psS.tile([128, 1, BH], F32, tag="sm")
                nc.tensor.matmul(bc_ps[:, 0, :], ones1b_sb[0:1, :],
                                 recip_sb[:, h0:h0 + BH], start=True, stop=True)
                nc.vector.tensor_copy(bc_sb[:, :, h0:h0 + BH], bc_ps[:, :, :])

            def tail_ctx_mm(h0):
                st = state[h0 // BH]
                if "ctx" in skip:
                    return
                ctx_ps = psS.tile([128, CC, BH], F32, tag="sm")
                st["ctx"] = ctx_ps
                for b in range(BH):
                    for cc in range(CC):
                        nc.tensor.matmul(
                            ctx_ps[:, cc, b:b + 1],
                            fs_sb[:, h0 + b, cc * 128:(cc + 1) * 128],
                            E_sb[:, h0 + b:h0 + b + 1],
                            start=True, stop=True)

            def tail_ctx_evac(h0):
                if "ctx" in skip:
                    nc.vector.memset(ctxT_sb[:, :, h0:h0 + BH], 0.01)
                    return
                ctx_ps = state[h0 // BH]["ctx"]
                # evac with the 1/s scale folded in
                nc.vector.tensor_tensor(
                    ctxT_sb[:, :, h0:h0 + BH], ctx_ps[:, :, :],
                    bc_sb[:, :, h0:h0 + BH].broadcast_to((128, CC, BH)),
                    op=ALU.mult)

            def tail_gates_ih(h0):
                if "gates" in skip:
                    return
                g_ps = state[h0 // BH]["g"]
                for mi in range(12, 24):
                    col = ((mi - 12) % 4) * 128 + (0 if mi < 16 else
                                                   512 if mi < 20 else 1024)
                    for kc in range(CC):
                        nc.tensor.matmul(
                            g_ps[:, mi, :], wih_sb[:, kc, col:col + 128],
                            ctxT_sb[:, kc, h0:h0 + BH],
                            start=(kc == 0), stop=False)
                    boff = (mi - 12) * 128 if mi < 20 else 1024 + (mi - 20) * 128
                    nc.tensor.matmul(
                        g_ps[:, mi, :], brow_sb[0:1, boff:boff + 128],
                        ones1b_sb[0:1, 0:BH], start=False, stop=True)
                # rz preactivation = hh part + ih part (the hh evac runs
                # early, off-chain: its matmuls finished at phase start)
                nc.vector.tensor_copy(g2_sb[:, :, h0:h0 + BH], g_ps[:, 0:8, :])
                nc.vector.tensor_tensor(g2_sb[:, :, h0:h0 + BH],
                                        g2_sb[:, :, h0:h0 + BH],
                                        g_ps[:, 12:20, :], op=ALU.add)
                nc.scalar.activation(rz_sb[:, :, h0:h0 + BH],
                                     g2_sb[:, :, h0:h0 + BH], AF.Tanh,
                                     scale=0.5)

            def tail_n_h(h0, k):
                if "gates" in skip:
                    nc.vector.memset(hist[:, :, k % 16, h0:h0 + BH], 0.01)
                    return
                g_ps = state[h0 // BH]["g"]
                hprev = hist[:, :, (k - 1) % 16, h0:h0 + BH]
                t1 = t1_sb[:, :, h0:h0 + BH]
                n = n_sb[:, :, h0:h0 + BH]
                # n = tanh(0.5*(A' + (t_r+1)*gh_n)), A' = 2*(W_ih,n ctx + b)
                nc.vector.scalar_tensor_tensor(
                    t1, rz_sb[:, 0:4, h0:h0 + BH], 1.0, g_ps[:, 8:12, :],
                    ALU.add, ALU.mult)
                nc.vector.tensor_tensor(t1, t1, g_ps[:, 20:24, :], op=ALU.add)
                nc.scalar.activation(n, t1, AF.Tanh, scale=0.5)
                # h' = n + 0.5*(t_z+1)*(h-n)
                nc.vector.tensor_tensor(t1, hprev, n, op=ALU.subtract)
                nc.vector.scalar_tensor_tensor(
                    t1, rz_sb[:, 4:8, h0:h0 + BH], 1.0, t1, ALU.add, ALU.mult)
                nc.vector.scalar_tensor_tensor(
                    hist[:, :, k % 16, h0:h0 + BH], t1, 0.5, n,
                    ALU.mult, ALU.add)

            def emit_phase(h0, k, tail_h0, tail_k, gen_items):
                """z-phase for half h0 at step k, with the other half's tail
                (for step tail_k) interleaved at fixed points."""
                me = state[h0 // BH] = {}
                do_tail = tail_k >= 0 and "tail" not in skip
                emit_hp(h0, k)
                emit_gates_hh(h0, k)
                e_ps = psS.tile([128, HC, BH], F32, tag="e", bufs=1)
                me["e"] = e_ps
                if do_tail:
                    # softmax + ctx matmuls first: the exp->1/s->ctx->gates
                    # chain is ~4.5us and must finish before rz (after z2)
                    tail_softmax(tail_h0)
                    tail_ctx_mm(tail_h0)
                zts = [emit_z(h0, 0, split=True)]
                zts.append(emit_z(h0, 1))
                if do_tail:
                    tail_ctx_evac(tail_h0)
                zts.append(emit_z(h0, 2))
                if do_tail:
                    tail_gates_ih(tail_h0)
                emit_e(zts[0], 0, e_ps, h0)
                emit_e(zts[1], 1, e_ps, h0)
                if do_tail:
                    tail_n_h(tail_h0, tail_k)
                zts.append(emit_z(h0, 3, split=True))
                emit_e(zts[2], 2, e_ps, h0)
                gen_work = [emit_gen_mm(*gi) + gi for gi in gen_items]
                emit_e(zts[3], 3, e_ps, h0, half=0)
                emit_e(zts[3], 3, e_ps, h0, half=1)
                # gen evacuations land in the boundary gap while DVE runs
                # the next phase's first z-add
                for (o_ps, m, k0, nk, off, width) in gen_work:
                    emit_gen_out(o_ps, m, k0, nk, off, width)

            def emit_tail(h0, k):
                tail_softmax(h0)
                tail_ctx_mm(h0)
                tail_ctx_evac(h0)
                tail_gates_ih(h0)
                tail_n_h(h0, k)

            from contextlib import ExitStack
            for rep in range(reps):
                if rep > 0:
                    nc.vector.memset(hist[:, :, 15, :], 0.0)
                state[0] = {}
                state[1] = {}
                with ExitStack() as scan_pools:
                    zp = scan_pools.enter_context(
                        tc.tile_pool(name=f"zp{rep}", bufs=6))
                    gop = scan_pools.enter_context(
                        tc.tile_pool(name=f"gop{rep}", bufs=3))
                    psS = scan_pools.enter_context(
                        tc.tile_pool(name=f"psS{rep}", bufs=2, space="PSUM"))
                    psG = scan_pools.enter_context(
                        tc.tile_pool(name=f"psG{rep}", bufs=2, space="PSUM"))
                    psO = scan_pools.enter_context(
                        tc.tile_pool(name=f"psO{rep}", bufs=3, space="PSUM"))
                    for k in range(L):
                        gi = [] if "gen" in skip else GEN_SCHED[k]
                        # a group's first emission step: the B half's h for
                        # the group's last step lands mid-A-phase; emitting
                        # its chunks in the A phase would head-of-line block
                        # PE behind that write, so they all go to the B phase
                        if any(k0 + nk == k for (k0, nk, _, _) in gi):
                            ga, gb = [], gi
                        else:
                            ga, gb = gi[:len(gi) // 2], gi[len(gi) // 2:]
                        emit_phase(0, k, BH, k - 1, ga)
                        emit_phase(BH, k, 0, k, gb)
                    if "tail" not in skip:
                        emit_tail(BH, L - 1)

                # ---- generator tail: rows for steps 24..25, on wide pools
                #      so the 14 chunks pipeline instead of serializing ----
                with (
                    tc.tile_pool(name=f"gopD{rep}", bufs=6) as gopD,
                    tc.tile_pool(name=f"psD{rep}", bufs=6, space="PSUM") as psD,
                ):
                    gop, psO = gopD, psD
                    for i, (k0, nk, off, width) in enumerate(
                            [] if "gen" in skip else GEN_TAIL):
                        o_ps, m = emit_gen_mm(k0, nk, off, width)
                        emit_gen_out(o_ps, m, k0, nk, off, width,
                                     on_act=(i % 2 == 0),
                                     ring=(nc.sync if i % 2 == 0 else
                                           nc.scalar))

    nc.finalize()
    return nc


def _get_graph():
    if not hasattr(_get_graph, "_nc"):
        _get_graph._nc = build_graph()
    return _get_graph._nc


def make_in_maps(feats, text_length, W_i2h, W_h2h, b_h2h, W_score, W_ih, W_hh,
                 b_ih, b_hh, W_gen, b_gen):
    bf = ml_dtypes.bfloat16
    feats = np.asarray(feats, np.float32)

    wi2h = np.ascontiguousarray(np.asarray(W_i2h, np.float32).T).astype(bf)
    wh2h = np.ascontiguousarray(np.asarray(W_h2h, np.float32).T).astype(bf)
    bh2hr = np.asarray(b_h2h, np.float32)[None, :].astype(bf)
    wcol = np.ascontiguousarray(
        np.asarray(W_score, np.float32)[0].reshape(HC, 128).T).astype(bf)
    # n-gate input weights/bias doubled: n = tanh(0.5*(A' + (t_r+1)*gh_n))
    wih_f = np.ascontiguousarray(np.asarray(W_ih, np.float32).T).copy()
    wih_f[:, 2 * H:] *= 2.0
    wih = wih_f.astype(bf)
    whh = np.ascontiguousarray(np.asarray(W_hh, np.float32).T).astype(bf)
    b_ih = np.asarray(b_ih, np.float32)
    b_hh = np.asarray(b_hh, np.float32)
    brow1 = np.concatenate([b_ih[:2 * H] + b_hh[:2 * H],
                            2.0 * b_ih[2 * H:], b_hh[2 * H:]])
    brow = brow1[None, :].astype(bf)
    wgen = np.ascontiguousarray(np.asarray(W_gen, np.float32).T).astype(bf)
    bgen = np.asarray(b_gen, np.float32)[None, :].astype(bf)

    in_maps = []
    for c in range(NCORES):
        fsh = feats[:, c * B:(c + 1) * B, :]                     # [T, B, C]
        featsT = np.ascontiguousarray(fsh.transpose(2, 1, 0)).astype(bf)
        featsS = np.ascontiguousarray(fsh.transpose(1, 0, 2)).astype(bf)
        in_maps.append({
            "featsT": featsT, "featsS": featsS, "wi2h": wi2h, "wh2h": wh2h,
            "bh2hr": bh2hr, "wcol": wcol, "wih": wih, "whh": whh,
            "brow": brow, "wgen": wgen, "bgen": bgen,
        })

    return in_maps


def kernel(**inputs):
    nc = _get_graph()
    in_maps = make_in_maps(**inputs)
    res = run_bass_kernel_spmd(nc, in_maps, core_ids=list(range(NCORES)))
    return np.concatenate(
        [res.results[c]["out"].transpose(1, 0, 2).reshape(B * L, NCLS)
         for c in range(NCORES)],
        axis=0)


# revision 27
# speedup vs baseline: 177.3471x; 1.0018x over previous
"""Bass/Trainium2 kernel for the attention-decoder problem (v3.1).

Data-parallel over batch: 8 cores x 32 batch each. Per core, a 26-step
Bahdanau-attention + GRU scan over T=128 encoder positions, then a
generator GEMM [832, 512] @ [512, 6736].

Structure:
- 2-way software pipeline over half-batches (16+16): half B runs half a
  step behind half A, so B's attention z-phase (Act-engine tanh, the
  per-step floor) overlaps A's serial tail (softmax, context, GRU) and
  vice versa; the Act engine stays ~continuously busy.
- gates are computed TRANSPOSED: stationary = W 128x128 chunks, moving =
  ctx^T/h^T [128, 16]; preactivations land [3H | b] in PSUM, gate math
  runs fully partition-parallel, and h' lands directly in [h | b] -- no
  transposes anywhere. The W_hh half of the gate GEMM only needs
  h(k-1), so it is issued at the start of the previous covering phase,
  off the critical chain; the W_ih half follows the context.
- context is computed TRANSPOSED: per (b, c-chunk) one matmul with
  stationary = feats_b chunk [t, c] and moving = the UNNORMALIZED
  exp(e) column; 1/s is broadcast across partitions via a k=1 matmul
  (computed in parallel with the context matmuls) and folded into the
  context PSUM->SBUF evacuation multiply.
- e^T accumulates per z-chunk into 4 PSUM columns right after each
  tanh; partial sums run early so only the last chunk's dot + one add
  sit on the critical chain.
- sigmoid(x) = 0.5 tanh(x/2) + 0.5 is algebraically folded so no gate
  fix-up op sits on the chain: with A' = 2*(W_ih,n ctx + b_ih,n)
  (weights pre-doubled on the host), n = tanh(0.5*(A' + (t_r+1)*gh_n))
  and h' = n + 0.5*(t_z+1)*(h-n), via scalar_tensor_tensor.
- all biases ride into PSUM as k=1 matmuls with stationary = bias-row.
- generator PSUM->SBUF copies run on the Act engine, queued exactly in
  the inter-phase boundary gap (while DVE runs the next z-add).
"""

import numpy as np
import ml_dtypes

import concourse.bass as bass
import concourse.mybir as mybir
import concourse.tile as tile
from concourse import bacc
from concourse.bass_utils import run_bass_kernel_spmd

BF16 = mybir.dt.bfloat16
F32 = mybir.dt.float32
AF = mybir.ActivationFunctionType
ALU = mybir.AluOpType

T, BFULL, C = 128, 256, 512
H, L, NCLS = 512, 26, 6736
NCORES = 8
B = BFULL // NCORES          # 32 per core
BH = B // 2                  # 16 per pipeline half
HC = H // 128                # 4 h-chunks
CC = C // 128                # 4 c-chunks
G3 = 3 * H                   # 1536
NSUB = 512                   # class-chunk width (one PSUM bank of f32)
NSUBS = [(i * NSUB, min(NSUB, NCLS - i * NSUB))
         for i in range((NCLS + NSUB - 1) // NSUB)]     # 14 chunks


def _gen_schedule():
    """sched[k] = list of (k0, nk, off, width) generator sub-GEMMs emitted
    during scan step k (group g: steps [4g, 4g+4), emitted over steps
    4g+4..4g+7, or 24..25 for g=5; g=6 runs after the scan)."""
    sched = {k: [] for k in list(range(L)) + [-1]}
    for g in range(6):
        k0, nk = 4 * g, 4
        if g < 5:
            # first step's chunks run in the B phase only (the other half's
            # h for the group's last step lands mid-A-phase); with the
            # 16-deep hist ring this is safe and keeps every boundary fed
            steps = [4 * g + 4] * 3 + [4 * g + 5] * 4 + [4 * g + 6] * 4 \
                + [4 * g + 7] * 3
        else:
            # last 4 chunks go to the drain (slots 4-7 are old there; they
            # run on the drain's 6 banks DURING the final tail's serial
            # chain, which otherwise leaves all engines idle)
            steps = [24] * 5 + [25] * 5 + [-1] * 4
        for i, (off, width) in enumerate(NSUBS):
            sched[steps[i]].append((k0, nk, off, width))
    return sched

GEN_SCHED = _gen_schedule()
GEN_TAIL = GEN_SCHED.pop(-1) + [(24, 2, off, width) for (off, width) in NSUBS]


def build_graph(reps=1, skip=()):
    nc = bacc.Bacc(None, target_bir_lowering=False, debug=False)

    # ---- DRAM parameters (per-core shard shapes) ----
    featsT = nc.declare_dram_parameter("featsT", [C, B, T], BF16, isOutput=False)
    featsS = nc.declare_dram_parameter("featsS", [B, T, C], BF16, isOutput=False)
    wi2h = nc.declare_dram_parameter("wi2h", [C, H], BF16, isOutput=False)
    wh2h = nc.declare_dram_parameter("wh2h", [H, H], BF16, isOutput=False)
    bh2hr = nc.declare_dram_parameter("bh2hr", [1, H], BF16, isOutput=False)
    wcol = nc.declare_dram_parameter("wcol", [128, HC], BF16, isOutput=False)
    wih = nc.declare_dram_parameter("wih", [C, G3], BF16, isOutput=False)
    whh = nc.declare_dram_parameter("whh", [H, G3], BF16, isOutput=False)
    brow = nc.declare_dram_parameter("brow", [1, 4 * H], BF16, isOutput=False)
    wgen = nc.declare_dram_parameter("wgen", [H, NCLS], BF16, isOutput=False)
    bgen = nc.declare_dram_parameter("bgen", [1, NCLS], BF16, isOutput=False)
    out = nc.declare_dram_parameter("out", [L, B, NCLS], F32, isOutput=True)

    with tile.TileContext(nc) as tc:
        with tc.tile_pool(name="pp", bufs=1) as pp:
            # ---- persistent SBUF ----
            P_sb = pp.tile([128, HC, T, B], BF16)      # i2h proj, [h | t, b]
            fs_sb = pp.tile([128, B, C], BF16)         # feats [t | b, c]
            wh2h_sb = pp.tile([128, HC, H], BF16)
            bh2hr_sb = pp.tile([1, H], BF16)
            wcol_sb = pp.tile([128, HC], BF16)
            wih_sb = pp.tile([128, CC, G3], BF16)
            whh_sb = pp.tile([128, HC, G3], BF16)
            brow_sb = pp.tile([1, 4 * H], BF16)
            wgen_sb = pp.tile([128, HC, NCLS], BF16)
            # h^T ring buffer, slot-major: slot k%16 holds h_k^T [h | b].
            # 16-deep so generator reads of old slots never make the h'
            # write wait (WAR hazard with a lagging gen group).
            hist = pp.tile([128, HC, 16, B], BF16)
            hp_sb = pp.tile([128, HC, B], BF16)        # hp^T evac
            ctxT_sb = pp.tile([128, CC, B], BF16)      # ctx^T evac
            et_sb = pp.tile([128, B], F32)             # e^T running sum
            E_sb = pp.tile([128, B], BF16)             # exp(e^T), unnormalized
            recip_sb = pp.tile([1, B], BF16)           # 1/s
            bc_sb = pp.tile([128, 1, B], F32)          # 1/s bcast across t
            g2_sb = pp.tile([128, 8, B], F32)          # rz pre-act (hh+ih)
            rz_sb = pp.tile([128, 8, B], BF16)         # raw tanh(0.5 rz)
            n_sb = pp.tile([128, HC, B], F32)          # n gate
            t1_sb = pp.tile([128, HC, B], F32)         # gate temp
            wi2h_pre = pp.tile([128, CC, H], BF16)
            ones128_sb = pp.tile([128, 1], BF16)
            ones1b_sb = pp.tile([1, 128], BF16)
            bgen_sb = pp.tile([1, NCLS], BF16)

            # ---- constant / weight loads, spread over 4 DMA rings so
            #      descriptor issue (~0.6us per dma_start per ring) overlaps ----
            nc.vector.memset(ones128_sb[:, :], 1.0)
            nc.vector.memset(ones1b_sb[:, :], 1.0)
            nc.vector.memset(hist[:, :, 15, :], 0.0)  # h_{-1} = 0 at slot 15
            for kc in range(CC):   # stationary for the prologue GEMM
                nc.gpsimd.dma_start(wi2h_pre[:, kc, :],
                                    wi2h[kc * 128:(kc + 1) * 128, :])
            for kc in range(HC):   # first use: hp at scan start
                nc.gpsimd.dma_start(wh2h_sb[:, kc, :],
                                    wh2h[kc * 128:(kc + 1) * 128, :])
            nc.gpsimd.dma_start(bh2hr_sb[:, :], bh2hr[:, :])
            nc.gpsimd.dma_start(wcol_sb[:, :], wcol[:, :])
            nc.gpsimd.dma_start(brow_sb[:, :], brow[:, :])
            nc.gpsimd.dma_start(bgen_sb[:, :], bgen[:, :])
            # big streams are scheduler-gated so their data transfers don't
            # contend with the prologue GEMM's featsT feed
            with tc.tile_wait_until(0.014):
                for b0 in range(0, B, 4):   # first use: ctx at ~60us
                    nc.gpsimd.dma_start(
                        fs_sb[:, b0:b0 + 4, :],
                        featsS[b0:b0 + 4, :, :].rearrange("b t c -> t b c"))
            with tc.tile_wait_until(0.030):  # first use ~58us (gates-hh k0)
                for kc in range(HC):
                    nc.gpsimd.dma_start(whh_sb[:, kc, :],
                                        whh[kc * 128:(kc + 1) * 128, :])
                for kc in range(CC):
                    nc.gpsimd.dma_start(wih_sb[:, kc, :],
                                        wih[kc * 128:(kc + 1) * 128, :])
            with tc.tile_wait_until(0.045):  # first use ~140us (gen step 4)
                for kc in range(HC):
                    nc.gpsimd.dma_start(wgen_sb[:, kc, :],
                                        wgen[kc * 128:(kc + 1) * 128, :])

            # ---- prologue: P^T = W_i2h^T @ feats^T, laid out [h | t, b] ----
            with (
                tc.tile_pool(name="ftp", bufs=2) as fp,
                tc.tile_pool(name="psP", bufs=4, space="PSUM") as psP,
            ):
                wi2h_sb = wi2h_pre
                # iterate over 4-sample groups (b-major) so half A's P is
                # complete at the halfway point and the scan starts early
                rings = (nc.sync, nc.scalar, nc.sync, nc.scalar)
                for bs in range(8):
                    ft_bs = fp.tile([128, CC, 4, T], BF16, tag="ft", bufs=5)
                    for kc in range(CC):
                        rings[kc].dma_start(
                            ft_bs[:, kc, :, :],
                            featsT[kc * 128:(kc + 1) * 128,
                                   bs * 4:(bs + 1) * 4, :])
                    for mp in range(0, HC, 2):
                        pps = [psP.tile([128, 512], F32, tag="pp",
                                        name=f"ppsum{q}") for q in (mp, mp + 1)]
                        for kc in range(CC):
                            for mc, ppsum in zip((mp, mp + 1), pps):
                                nc.tensor.matmul(
                                    ppsum[:, :],
                                    wi2h_sb[:, kc, mc * 128:(mc + 1) * 128],
                                    ft_bs[:, kc, :, :],
                                    start=(kc == 0), stop=(kc == CC - 1))
                        for mc, ppsum in zip((mp, mp + 1), pps):
                            dst = P_sb[:, mc, :, bs * 4:(bs + 1) * 4]
                            src_ap = ppsum[:, :].rearrange(
                                "p (b t) -> p t b", b=4)
                            if mc % 2 == 0:
                                nc.vector.tensor_copy(dst, src_ap)
                            else:
                                nc.scalar.activation(dst, src_ap, AF.Copy)

            state = [None, None]   # per-half carry between phases
            genct = [0]            # alternate gen-copy engine

            def emit_hp(h0, k):
                """hp^T = W_h2h^T h^T + b -> PSUM; evac chunk 0 first so the
                first z-add can start before the rest lands."""
                hp_ps = psS.tile([128, HC, BH], F32, tag="sm")
                for mc in range(HC):
                    for kc in range(HC):
                        nc.tensor.matmul(
                            hp_ps[:, mc, :],
                            wh2h_sb[:, kc, mc * 128:(mc + 1) * 128],
                            hist[:, kc, (k - 1) % 16, h0:h0 + BH],
                            start=(kc == 0), stop=False)
                    nc.tensor.matmul(
                        hp_ps[:, mc, :],
                        bh2hr_sb[0:1, mc * 128:(mc + 1) * 128],
                        ones1b_sb[0:1, 0:BH], start=False, stop=True)
                nc.vector.tensor_copy(hp_sb[:, 0, h0:h0 + BH], hp_ps[:, 0, :])
                nc.vector.tensor_copy(hp_sb[:, 1:, h0:h0 + BH], hp_ps[:, 1:, :])

            def emit_gates_hh(h0, k):
                """W_hh part of the gate GEMM + n_h bias; only needs h(k-1),
                so it runs at the covering phase's start, off-chain.
                g tile regions: 0-7 rz(hh), 8-11 n_h(+b_hh,n),
                12-19 rz(ih)+b_rz, 20-23 2*(n_i+b_ih,n)."""
                g_ps = psG.tile([128, 24, BH], F32, tag="g")
                state[h0 // BH]["g"] = g_ps
                if "gates" in skip:
                    return
                hT = hist[:, :, (k - 1) % 16, :]
                for mi in range(12):
                    col = (mi % 4) * 128 + (0 if mi < 4 else
                                            512 if mi < 8 else 1024)
                    for kc in range(HC):
                        nc.tensor.matmul(
                            g_ps[:, mi, :], whh_sb[:, kc, col:col + 128],
                            hT[:, kc, h0:h0 + BH],
                            start=(kc == 0), stop=(mi < 8 and kc == HC - 1))
                    if mi >= 8:   # + b_hh,n
                        boff = 1536 + (mi - 8) * 128
                        nc.tensor.matmul(
                            g_ps[:, mi, :], brow_sb[0:1, boff:boff + 128],
                            ones1b_sb[0:1, 0:BH], start=False, stop=True)

            def emit_z(h0, hc, split=False):
                z = zp.tile([128, T, BH], BF16, tag="z")
                spans = ((0, T // 2), (T // 2, T)) if split else ((0, T),)
                for (t0, t1) in spans:
                    nc.vector.tensor_tensor(
                        z[:, t0:t1, :], P_sb[:, hc, t0:t1, h0:h0 + BH],
                        hp_sb[:, hc:hc + 1, h0:h0 + BH]
                        .broadcast_to((128, t1 - t0, BH)),
                        op=ALU.add)
                    if "tanh" not in skip:
                        nc.scalar.activation(z[:, t0:t1, :], z[:, t0:t1, :],
                                             AF.Tanh)
                return z

            def emit_e(z, hc, e_ps, h0, half=None):
                """one e^T column set from z chunk hc + running et update.
                half=0/1 restricts to t rows [0,64)/[64,128) (chunk-3 split
                so the first half overlaps the second half's tanh)."""
                if "e" in skip:
                    if hc == 0:
                        nc.vector.memset(et_sb[:, h0:h0 + BH], 0.5)
                    return
                r0, r1 = (0, T) if half is None else (half * 64, half * 64 + 64)
                for b in range(BH):
                    nc.tensor.matmul(
                        e_ps[r0:r1, hc, b:b + 1], z[:, r0:r1, b],
                        wcol_sb[:, hc:hc + 1], start=True, stop=True)
                if hc == 0:
                    nc.vector.tensor_copy(et_sb[:, h0:h0 + BH], e_ps[:, 0, :])
                elif hc == 1:
                    nc.vector.tensor_tensor(et_sb[:, h0:h0 + BH],
                                            et_sb[:, h0:h0 + BH],
                                            e_ps[:, 1, :], op=ALU.add)
                elif hc == 2:
                    nc.vector.tensor_tensor(et_sb[:, h0:h0 + BH],
                                            et_sb[:, h0:h0 + BH],
                                            e_ps[:, 2, :], op=ALU.add)
                elif hc == 3:
                    nc.vector.tensor_tensor(et_sb[r0:r1, h0:h0 + BH],
                                            et_sb[r0:r1, h0:h0 + BH],
                                            e_ps[r0:r1, 3, :], op=ALU.add)

            def emit_gen_mm(k0, nk, off, width):
                m = B * nk
                s0 = k0 % 16
                o_ps = psO.tile([128, NSUB], F32, tag="gen")
                for hc in range(HC):
                    nc.tensor.matmul(
                        o_ps[:m, 0:width],
                        hist[:, hc, s0:s0 + nk, :],
                        wgen_sb[:, hc, off:off + width],
                        start=(hc == 0), stop=False)
                nc.tensor.matmul(
                    o_ps[:m, 0:width], ones1b_sb[0:1, 0:m],
                    bgen_sb[0:1, off:off + width], start=False, stop=True)
                return o_ps, m

            def emit_gen_out(o_ps, m, k0, nk, off, width, on_act=True,
                             ring=None):
                o_sb = gop.tile([128, NSUB], F32, tag="ob")
                if on_act:
                    nc.scalar.activation(o_sb[:m, 0:width], o_ps[:m, 0:width],
                                         AF.Copy)
                else:
                    nc.vector.tensor_copy(o_sb[:m, 0:width], o_ps[:m, 0:width])
                if ring is None:
                    genct[0] += 1
                    ring = nc.sync if genct[0] % 2 == 0 else nc.scalar
                ring.dma_start(
                    out[k0:k0 + nk, :, off:off + width], o_sb[:m, 0:width])

            # ---- tail sub-blocks for half at h0 (its z-phase ran last
            #      phase; e_ps columns 0-2 are already summed in et_sb) ----
            def tail_softmax(h0):
                st = state[h0 // BH]
                nc.scalar.activation(E_sb[:, h0:h0 + BH], et_sb[:, h0:h0 + BH],
                                     AF.Exp)
                s_ps = psS.tile([1, BH], F32, tag="sm")
                nc.tensor.matmul(s_ps[:, :], ones128_sb[:, :],
                                 E_sb[:, h0:h0 + BH], start=True, stop=True)
                with nc.allow_low_precision(reason="1/s bf16; ctx is bf16"):
                    nc.vector.reciprocal(recip_sb[:, h0:h0 + BH], s_ps[:, :])
                bc_ps = psS.tile([128, 1, BH], F32, tag="sm")
                nc.tensor.matmul(bc_ps[:, 0, :], ones1b_sb[0:1, :],
                                 recip_sb[:, h0:h0 + BH], start=True, stop=True)
                nc.vector.tensor_copy(bc_sb[:, :, h0:h0 + BH], bc_ps[:, :, :])

            def tail_ctx_mm(h0):
                st = state[h0 // BH]
                if "ctx" in skip:
                    return
                ctx_ps = psS.tile([128, CC, BH], F32, tag="sm")
                st["ctx"] = ctx_ps
                for b in range(BH):
                    for cc in range(CC):
                        nc.tensor.matmul(
                            ctx_ps[:, cc, b:b + 1],
                            fs_sb[:, h0 + b, cc * 128:(cc + 1) * 128],
                            E_sb[:, h0 + b:h0 + b + 1],
                            start=True, stop=True)

            def tail_ctx_evac(h0):
                if "ctx" in skip:
                    nc.vector.memset(ctxT_sb[:, :, h0:h0 + BH], 0.01)
                    return
                ctx_ps = state[h0 // BH]["ctx"]
                # evac with the 1/s scale folded in
                nc.vector.tensor_tensor(
                    ctxT_sb[:, :, h0:h0 + BH], ctx_ps[:, :, :],
                    bc_sb[:, :, h0:h0 + BH].broadcast_to((128, CC, BH)),
                    op=ALU.mult)

            def tail_gates_ih(h0):
                if "gates" in skip:
                    return
                g_ps = state[h0 // BH]["g"]
                for mi in range(12, 24):
                    col = ((mi - 12) % 4) * 128 + (0 if mi < 16 else
                                                   512 if mi < 20 else 1024)
                    for kc in range(CC):
                        nc.tensor.matmul(
                            g_ps[:, mi, :], wih_sb[:, kc, col:col + 128],
                            ctxT_sb[:, kc, h0:h0 + BH],
                            start=(kc == 0), stop=False)
                    boff = (mi - 12) * 128 if mi < 20 else 1024 + (mi - 20) * 128
                    nc.tensor.matmul(
                        g_ps[:, mi, :], brow_sb[0:1, boff:boff + 128],
                        ones1b_sb[0:1, 0:BH], start=False, stop=True)
                # rz preactivation = hh part + ih part (the hh evac runs
                # early, off-chain: its matmuls finished at phase start)
                nc.vector.tensor_copy(g2_sb[:, :, h0:h0 + BH], g_ps[:, 0:8, :])
                nc.vector.tensor_tensor(g2_sb[:, :, h0:h0 + BH],
                                        g2_sb[:, :, h0:h0 + BH],
                                        g_ps[:, 12:20, :], op=ALU.add)
                nc.scalar.activation(rz_sb[:, :, h0:h0 + BH],
                                     g2_sb[:, :, h0:h0 + BH], AF.Tanh,
                                     scale=0.5)

            def tail_n_h(h0, k):
                if "gates" in skip:
                    nc.vector.memset(hist[:, :, k % 16, h0:h0 + BH], 0.01)
                    return
                g_ps = state[h0 // BH]["g"]
                hprev = hist[:, :, (k - 1) % 16, h0:h0 + BH]
                t1 = t1_sb[:, :, h0:h0 + BH]
                n = n_sb[:, :, h0:h0 + BH]
                # n = tanh(0.5*(A' + (t_r+1)*gh_n)), A' = 2*(W_ih,n ctx + b)
                nc.vector.scalar_tensor_tensor(
                    t1, rz_sb[:, 0:4, h0:h0 + BH], 1.0, g_ps[:, 8:12, :],
                    ALU.add, ALU.mult)
                nc.vector.tensor_tensor(t1, t1, g_ps[:, 20:24, :], op=ALU.add)
                nc.scalar.activation(n, t1, AF.Tanh, scale=0.5)
                # h' = n + 0.5*(t_z+1)*(h-n)
                nc.vector.tensor_tensor(t1, hprev, n, op=ALU.subtract)
                nc.vector.scalar_tensor_tensor(
                    t1, rz_sb[:, 4:8, h0:h0 + BH], 1.0, t1, ALU.add, ALU.mult)
                nc.vector.scalar_tensor_tensor(
                    hist[:, :, k % 16, h0:h0 + BH], t1, 0.5, n,
                    ALU.mult, ALU.add)

            def emit_phase(h0, k, tail_h0, tail_k, gen_items):
                """z-phase for half h0 at step k, with the other half's tail
                (for step tail_k) interleaved at fixed points."""
                me = state[h0 // BH] = {}
                do_tail = tail_k >= 0 and "tail" not in skip
                emit_hp(h0, k)
                emit_gates_hh(h0, k)
                e_ps = psS.tile([128, HC, BH], F32, tag="e", bufs=1)
                me["e"] = e_ps
                if do_tail:
                    # softmax + ctx matmuls first: the exp->1/s->ctx->gates
                    # chain is ~4.5us and must finish before rz (after z2)
                    tail_softmax(tail_h0)
                    tail_ctx_mm(tail_h0)
                zts = [emit_z(h0, 0, split=True)]
                zts.append(emit_z(h0, 1))
                if do_tail:
                    tail_ctx_evac(tail_h0)
                zts.append(emit_z(h0, 2))
                if do_tail:
                    tail_gates_ih(tail_h0)
                emit_e(zts[0], 0, e_ps, h0)
                emit_e(zts[1], 1, e_ps, h0)
                if do_tail:
                    tail_n_h(tail_h0, tail_k)
                zts.append(emit_z(h0, 3, split=True))
                emit_e(zts[2], 2, e_ps, h0)
                gen_work = [emit_gen_mm(*gi) + gi for gi in gen_items]
                emit_e(zts[3], 3, e_ps, h0, half=0)
                emit_e(zts[3], 3, e_ps, h0, half=1)
                # gen evacuations land in the boundary gap while DVE runs
                # the next phase's first z-add
                for (o_ps, m, k0, nk, off, width) in gen_work:
                    emit_gen_out(o_ps, m, k0, nk, off, width)

            def emit_tail(h0, k):
                tail_softmax(h0)
                tail_ctx_mm(h0)
                tail_ctx_evac(h0)
                tail_gates_ih(h0)
                tail_n_h(h0, k)

            from contextlib import ExitStack
            for rep in range(reps):
                if rep > 0:
                    nc.vector.memset(hist[:, :, 15, :], 0.0)
                state[0] = {}
                state[1] = {}
                with ExitStack() as scan_pools:
                    zp = scan_pools.enter_context(
                        tc.tile_pool(name=f"zp{rep}", bufs=6))
                    gop = scan_pools.enter_context(
                        tc.tile_pool(name=f"gop{rep}", bufs=3))
                    psS = scan_pools.enter_context(
                        tc.tile_pool(name=f"psS{rep}", bufs=2, space="PSUM"))
                    psG = scan_pools.enter_context(
                        tc.tile_pool(name=f"psG{rep}", bufs=2, space="PSUM"))
                    psO = scan_pools.enter_context(
                        tc.tile_pool(name=f"psO{rep}", bufs=3, space="PSUM"))
                    for k in range(L):
                        gi = [] if "gen" in skip else GEN_SCHED[k]
                        # a group's first emission step: the B half's h for
                        # the group's last step lands mid-A-phase; emitting
                        # its chunks in the A phase would head-of-line block
                        # PE behind that write, so they all go to the B phase
                        if any(k0 + nk == k for (k0, nk, _, _) in gi):
                            ga, gb = [], gi
                        else:
                            ga, gb = gi[:len(gi) // 2], gi[len(gi) // 2:]
                        emit_phase(0, k, BH, k - 1, ga)
                        emit_phase(BH, k, 0, k, gb)
                    if "tail" not in skip:
                        emit_tail(BH, L - 1)

                # ---- generator tail: rows for steps 24..25, on wide pools
                #      so the 14 chunks pipeline instead of serializing ----
                with (
                    tc.tile_pool(name=f"gopD{rep}", bufs=8) as gopD,
                    tc.tile_pool(name=f"psD{rep}", bufs=8, space="PSUM") as psD,
                ):
                    gop, psO = gopD, psD
                    for i, (k0, nk, off, width) in enumerate(
                            [] if "gen" in skip else GEN_TAIL):
                        o_ps, m = emit_gen_mm(k0, nk, off, width)
                        emit_gen_out(o_ps, m, k0, nk, off, width,
                                     on_act=(i % 2 == 0),
                                     ring=(nc.sync if i % 2 == 0 else
                                           nc.scalar))

    nc.finalize()
    return nc


def _get_graph():
    if not hasattr(_get_graph, "_nc"):
        _get_graph._nc = build_graph()
    return _get_graph._nc


def make_in_maps(feats, text_length, W_i2h, W_h2h, b_h2h, W_score, W_ih, W_hh,
                 b_ih, b_hh, W_gen, b_gen):
    bf = ml_dtypes.bfloat16
    feats = np.asarray(feats, np.float32)

    wi2h = np.ascontiguousarray(np.asarray(W_i2h, np.float32).T).astype(bf)
    wh2h = np.ascontiguousarray(np.asarray(W_h2h, np.float32).T).astype(bf)
    bh2hr = np.asarray(b_h2h, np.float32)[None, :].astype(bf)
    wcol = np.ascontiguousarray(
        np.asarray(W_score, np.float32)[0].reshape(HC, 128).T).astype(bf)
    # n-gate input weights/bias doubled: n = tanh(0.5*(A' + (t_r+1)*gh_n))
    wih_f = np.ascontiguousarray(np.asarray(W_ih, np.float32).T).copy()
    wih_f[:, 2 * H:] *= 2.0
    wih = wih_f.astype(bf)
    whh = np.ascontiguousarray(np.asarray(W_hh, np.float32).T).astype(bf)
    b_ih = np.asarray(b_ih, np.float32)
    b_hh = np.asarray(b_hh, np.float32)
    brow1 = np.concatenate([b_ih[:2 * H] + b_hh[:2 * H],
                            2.0 * b_ih[2 * H:], b_hh[2 * H:]])
    brow = brow1[None, :].astype(bf)
    wgen = np.ascontiguousarray(np.asarray(W_gen, np.float32).T).astype(bf)
    bgen = np.asarray(b_gen, np.float32)[None, :].astype(bf)

    in_maps = []
    for c in range(NCORES):
        fsh = feats[:, c * B:(c + 1) * B, :]                     # [T, B, C]
        featsT = np.ascontiguousarray(fsh.transpose(2, 1, 0)).astype(bf)
        featsS = np.ascontiguousarray(fsh.transpose(1, 0, 2)).astype(bf)
        in_maps.append({
            "featsT": featsT, "featsS": featsS, "wi2h": wi2h, "wh2h": wh2h,
            "bh2hr": bh2hr, "wcol": wcol, "wih": wih, "whh": whh,
            "brow": brow, "wgen": wgen, "bgen": bgen,
        })

    return in_maps


def kernel(**inputs):
    nc = _get_graph()
    in_maps = make_in_maps(**inputs)
    res = run_bass_kernel_spmd(nc, in_maps, core_ids=list(range(NCORES)))
    return np.concatenate(
        [res.results[c]["out"].transpose(1, 0, 2).reshape(B * L, NCLS)
         for c in range(NCORES)],
        axis=0)


# revision 28
# speedup vs baseline: 177.4228x; 1.0004x over previous
"""Bass/Trainium2 kernel for the attention-decoder problem (v3.1).

Data-parallel over batch: 8 cores x 32 batch each. Per core, a 26-step
Bahdanau-attention + GRU scan over T=128 encoder positions, then a
generator GEMM [832, 512] @ [512, 6736].

Structure:
- 2-way software pipeline over half-batches (16+16): half B runs half a
  step behind half A, so B's attention z-phase (Act-engine tanh, the
  per-step floor) overlaps A's serial tail (softmax, context, GRU) and
  vice versa; the Act engine stays ~continuously busy.
- gates are computed TRANSPOSED: stationary = W 128x128 chunks, moving =
  ctx^T/h^T [128, 16]; preactivations land [3H | b] in PSUM, gate math
  runs fully partition-parallel, and h' lands directly in [h | b] -- no
  transposes anywhere. The W_hh half of the gate GEMM only needs
  h(k-1), so it is issued at the start of the previous covering phase,
  off the critical chain; the W_ih half follows the context.
- context is computed TRANSPOSED: per (b, c-chunk) one matmul with
  stationary = feats_b chunk [t, c] and moving = the UNNORMALIZED
  exp(e) column; 1/s is broadcast across partitions via a k=1 matmul
  (computed in parallel with the context matmuls) and folded into the
  context PSUM->SBUF evacuation multiply.
- e^T accumulates per z-chunk into 4 PSUM columns right after each
  tanh; partial sums run early so only the last chunk's dot + one add
  sit on the critical chain.
- sigmoid(x) = 0.5 tanh(x/2) + 0.5 is algebraically folded so no gate
  fix-up op sits on the chain: with A' = 2*(W_ih,n ctx + b_ih,n)
  (weights pre-doubled on the host), n = tanh(0.5*(A' + (t_r+1)*gh_n))
  and h' = n + 0.5*(t_z+1)*(h-n), via scalar_tensor_tensor.
- all biases ride into PSUM as k=1 matmuls with stationary = bias-row.
- generator PSUM->SBUF copies run on the Act engine, queued exactly in
  the inter-phase boundary gap (while DVE runs the next z-add).
"""

import numpy as np
import ml_dtypes

import concourse.bass as bass
import concourse.mybir as mybir
import concourse.tile as tile
from concourse import bacc
from concourse.bass_utils import run_bass_kernel_spmd

BF16 = mybir.dt.bfloat16
F32 = mybir.dt.float32
AF = mybir.ActivationFunctionType
ALU = mybir.AluOpType

T, BFULL, C = 128, 256, 512
H, L, NCLS = 512, 26, 6736
NCORES = 8
B = BFULL // NCORES          # 32 per core
BH = B // 2                  # 16 per pipeline half
HC = H // 128                # 4 h-chunks
CC = C // 128                # 4 c-chunks
G3 = 3 * H                   # 1536
NSUB = 512                   # class-chunk width (one PSUM bank of f32)
NSUBS = [(i * NSUB, min(NSUB, NCLS - i * NSUB))
         for i in range((NCLS + NSUB - 1) // NSUB)]     # 14 chunks


def _gen_schedule():
    """sched[k] = list of (k0, nk, off, width) generator sub-GEMMs emitted
    during scan step k (group g: steps [4g, 4g+4), emitted over steps
    4g+4..4g+7, or 24..25 for g=5; g=6 runs after the scan)."""
    sched = {k: [] for k in list(range(L)) + [-1]}
    for g in range(6):
        k0, nk = 4 * g, 4
        if g < 5:
            # first step's chunks run in the B phase only (the other half's
            # h for the group's last step lands mid-A-phase); with the
            # 16-deep hist ring this is safe and keeps every boundary fed
            steps = [4 * g + 4] * 3 + [4 * g + 5] * 4 + [4 * g + 6] * 4 \
                + [4 * g + 7] * 3
        else:
            # last 4 chunks go to the drain (slots 4-7 are old there; they
            # run on the drain's 6 banks DURING the final tail's serial
            # chain, which otherwise leaves all engines idle)
            steps = [24] * 5 + [25] * 5 + [-1] * 4
        for i, (off, width) in enumerate(NSUBS):
            sched[steps[i]].append((k0, nk, off, width))
    return sched

GEN_SCHED = _gen_schedule()
GEN_TAIL = GEN_SCHED.pop(-1) + [(24, 2, off, width) for (off, width) in NSUBS]


def build_graph(reps=1, skip=()):
    nc = bacc.Bacc(None, target_bir_lowering=False, debug=False)

    # ---- DRAM parameters (per-core shard shapes) ----
    featsT = nc.declare_dram_parameter("featsT", [C, B, T], BF16, isOutput=False)
    featsS = nc.declare_dram_parameter("featsS", [B, T, C], BF16, isOutput=False)
    wi2h = nc.declare_dram_parameter("wi2h", [C, H], BF16, isOutput=False)
    wh2h = nc.declare_dram_parameter("wh2h", [H, H], BF16, isOutput=False)
    bh2hr = nc.declare_dram_parameter("bh2hr", [1, H], BF16, isOutput=False)
    wcol = nc.declare_dram_parameter("wcol", [128, HC], BF16, isOutput=False)
    wih = nc.declare_dram_parameter("wih", [C, G3], BF16, isOutput=False)
    whh = nc.declare_dram_parameter("whh", [H, G3], BF16, isOutput=False)
    brow = nc.declare_dram_parameter("brow", [1, 4 * H], BF16, isOutput=False)
    wgen = nc.declare_dram_parameter("wgen", [H, NCLS], BF16, isOutput=False)
    bgen = nc.declare_dram_parameter("bgen", [1, NCLS], BF16, isOutput=False)
    out = nc.declare_dram_parameter("out", [L, B, NCLS], F32, isOutput=True)

    with tile.TileContext(nc) as tc:
        with tc.tile_pool(name="pp", bufs=1) as pp:
            # ---- persistent SBUF ----
            P_sb = pp.tile([128, HC, T, B], BF16)      # i2h proj, [h | t, b]
            fs_sb = pp.tile([128, B, C], BF16)         # feats [t | b, c]
            wh2h_sb = pp.tile([128, HC, H], BF16)
            bh2hr_sb = pp.tile([1, H], BF16)
            wcol_sb = pp.tile([128, HC], BF16)
            wih_sb = pp.tile([128, CC, G3], BF16)
            whh_sb = pp.tile([128, HC, G3], BF16)
            brow_sb = pp.tile([1, 4 * H], BF16)
            wgen_sb = pp.tile([128, HC, NCLS], BF16)
            # h^T ring buffer, slot-major: slot k%16 holds h_k^T [h | b].
            # 16-deep so generator reads of old slots never make the h'
            # write wait (WAR hazard with a lagging gen group).
            hist = pp.tile([128, HC, 16, B], BF16)
            hp_sb = pp.tile([128, HC, B], BF16)        # hp^T evac
            ctxT_sb = pp.tile([128, CC, B], BF16)      # ctx^T evac
            et_sb = pp.tile([128, B], F32)             # e^T running sum
            E_sb = pp.tile([128, B], BF16)             # exp(e^T), unnormalized
            recip_sb = pp.tile([1, B], BF16)           # 1/s
            bc_sb = pp.tile([128, 1, B], F32)          # 1/s bcast across t
            g2_sb = pp.tile([128, 8, B], F32)          # rz pre-act (hh+ih)
            rz_sb = pp.tile([128, 8, B], BF16)         # raw tanh(0.5 rz)
            n_sb = pp.tile([128, HC, B], F32)          # n gate
            t1_sb = pp.tile([128, HC, B], F32)         # gate temp
            wi2h_pre = pp.tile([128, CC, H], BF16)
            ones128_sb = pp.tile([128, 1], BF16)
            ones1b_sb = pp.tile([1, 128], BF16)
            bgen_sb = pp.tile([1, NCLS], BF16)

            # ---- constant / weight loads, spread over 4 DMA rings so
            #      descriptor issue (~0.6us per dma_start per ring) overlaps ----
            nc.vector.memset(ones128_sb[:, :], 1.0)
            nc.vector.memset(ones1b_sb[:, :], 1.0)
            nc.vector.memset(hist[:, :, 15, :], 0.0)  # h_{-1} = 0 at slot 15
            for kc in range(CC):   # stationary for the prologue GEMM
                nc.gpsimd.dma_start(wi2h_pre[:, kc, :],
                                    wi2h[kc * 128:(kc + 1) * 128, :])
            for kc in range(HC):   # first use: hp at scan start
                nc.gpsimd.dma_start(wh2h_sb[:, kc, :],
                                    wh2h[kc * 128:(kc + 1) * 128, :])
            nc.gpsimd.dma_start(bh2hr_sb[:, :], bh2hr[:, :])
            nc.gpsimd.dma_start(wcol_sb[:, :], wcol[:, :])
            nc.gpsimd.dma_start(brow_sb[:, :], brow[:, :])
            nc.gpsimd.dma_start(bgen_sb[:, :], bgen[:, :])
            # big streams are scheduler-gated so their data transfers don't
            # contend with the prologue GEMM's featsT feed
            with tc.tile_wait_until(0.014):
                for b0 in range(0, B, 4):   # first use: ctx at ~60us
                    nc.gpsimd.dma_start(
                        fs_sb[:, b0:b0 + 4, :],
                        featsS[b0:b0 + 4, :, :].rearrange("b t c -> t b c"))
            with tc.tile_wait_until(0.030):  # first use ~58us (gates-hh k0)
                for kc in range(HC):
                    nc.gpsimd.dma_start(whh_sb[:, kc, :],
                                        whh[kc * 128:(kc + 1) * 128, :])
                for kc in range(CC):
                    nc.gpsimd.dma_start(wih_sb[:, kc, :],
                                        wih[kc * 128:(kc + 1) * 128, :])
            with tc.tile_wait_until(0.045):  # first use ~140us (gen step 4)
                for kc in range(HC):
                    nc.gpsimd.dma_start(wgen_sb[:, kc, :],
                                        wgen[kc * 128:(kc + 1) * 128, :])

            # ---- prologue: P^T = W_i2h^T @ feats^T, laid out [h | t, b] ----
            with (
                tc.tile_pool(name="ftp", bufs=2) as fp,
                tc.tile_pool(name="psP", bufs=4, space="PSUM") as psP,
            ):
                wi2h_sb = wi2h_pre
                # iterate over 4-sample groups (b-major) so half A's P is
                # complete at the halfway point and the scan starts early
                rings = (nc.sync, nc.scalar, nc.sync, nc.scalar)
                for bs in range(8):
                    ft_bs = fp.tile([128, CC, 4, T], BF16, tag="ft", bufs=5)
                    for kc in range(CC):
                        rings[kc].dma_start(
                            ft_bs[:, kc, :, :],
                            featsT[kc * 128:(kc + 1) * 128,
                                   bs * 4:(bs + 1) * 4, :])
                    for mp in range(0, HC, 2):
                        pps = [psP.tile([128, 512], F32, tag="pp",
                                        name=f"ppsum{q}") for q in (mp, mp + 1)]
                        for kc in range(CC):
                            for mc, ppsum in zip((mp, mp + 1), pps):
                                nc.tensor.matmul(
                                    ppsum[:, :],
                                    wi2h_sb[:, kc, mc * 128:(mc + 1) * 128],
                                    ft_bs[:, kc, :, :],
                                    start=(kc == 0), stop=(kc == CC - 1))
                        for mc, ppsum in zip((mp, mp + 1), pps):
                            dst = P_sb[:, mc, :, bs * 4:(bs + 1) * 4]
                            src_ap = ppsum[:, :].rearrange(
                                "p (b t) -> p t b", b=4)
                            if mc % 2 == 0:
                                nc.vector.tensor_copy(dst, src_ap)
                            else:
                                nc.scalar.activation(dst, src_ap, AF.Copy)

            state = [None, None]   # per-half carry between phases
            genct = [0]            # alternate gen-copy engine

            def emit_hp(h0, k):
                """hp^T = W_h2h^T h^T + b -> PSUM; evac chunk 0 first so the
                first z-add can start before the rest lands."""
                hp_ps = psS.tile([128, HC, BH], F32, tag="sm")
                for mc in range(HC):
                    for kc in range(HC):
                        nc.tensor.matmul(
                            hp_ps[:, mc, :],
                            wh2h_sb[:, kc, mc * 128:(mc + 1) * 128],
                            hist[:, kc, (k - 1) % 16, h0:h0 + BH],
                            start=(kc == 0), stop=False)
                    nc.tensor.matmul(
                        hp_ps[:, mc, :],
                        bh2hr_sb[0:1, mc * 128:(mc + 1) * 128],
                        ones1b_sb[0:1, 0:BH], start=False, stop=True)
                nc.vector.tensor_copy(hp_sb[:, 0, h0:h0 + BH], hp_ps[:, 0, :])
                nc.vector.tensor_copy(hp_sb[:, 1:, h0:h0 + BH], hp_ps[:, 1:, :])

            def emit_gates_hh(h0, k):
                """W_hh part of the gate GEMM + n_h bias; only needs h(k-1),
                so it runs at the covering phase's start, off-chain.
                g tile regions: 0-7 rz(hh), 8-11 n_h(+b_hh,n),
                12-19 rz(ih)+b_rz, 20-23 2*(n_i+b_ih,n)."""
                g_ps = psG.tile([128, 28, BH], F32, tag="g")
                state[h0 // BH]["g"] = g_ps
                if "gates" in skip:
                    return
                hT = hist[:, :, (k - 1) % 16, :]
                for mi in range(12):
                    col = (mi % 4) * 128 + (0 if mi < 4 else
                                            512 if mi < 8 else 1024)
                    for kc in range(HC):
                        nc.tensor.matmul(
                            g_ps[:, mi, :], whh_sb[:, kc, col:col + 128],
                            hT[:, kc, h0:h0 + BH],
                            start=(kc == 0), stop=(mi < 8 and kc == HC - 1))
                    if mi >= 8:   # + b_hh,n
                        boff = 1536 + (mi - 8) * 128
                        nc.tensor.matmul(
                            g_ps[:, mi, :], brow_sb[0:1, boff:boff + 128],
                            ones1b_sb[0:1, 0:BH], start=False, stop=True)

            def emit_z(h0, hc, split=False):
                z = zp.tile([128, T, BH], BF16, tag="z")
                spans = ((0, T // 2), (T // 2, T)) if split else ((0, T),)
                for (t0, t1) in spans:
                    nc.vector.tensor_tensor(
                        z[:, t0:t1, :], P_sb[:, hc, t0:t1, h0:h0 + BH],
                        hp_sb[:, hc:hc + 1, h0:h0 + BH]
                        .broadcast_to((128, t1 - t0, BH)),
                        op=ALU.add)
                    if "tanh" not in skip:
                        nc.scalar.activation(z[:, t0:t1, :], z[:, t0:t1, :],
                                             AF.Tanh)
                return z

            def emit_e(z, hc, e_ps, h0, half=None):
                """one e^T column set from z chunk hc + running et update.
                half=0/1 restricts to t rows [0,64)/[64,128) (chunk-3 split
                so the first half overlaps the second half's tanh)."""
                if "e" in skip:
                    if hc == 0:
                        nc.vector.memset(et_sb[:, h0:h0 + BH], 0.5)
                    return
                r0, r1 = (0, T) if half is None else (half * 64, half * 64 + 64)
                for b in range(BH):
                    nc.tensor.matmul(
                        e_ps[r0:r1, hc, b:b + 1], z[:, r0:r1, b],
                        wcol_sb[:, hc:hc + 1], start=True, stop=True)
                if hc == 0:
                    nc.vector.tensor_copy(et_sb[:, h0:h0 + BH], e_ps[:, 0, :])
                elif hc == 1:
                    nc.vector.tensor_tensor(et_sb[:, h0:h0 + BH],
                                            et_sb[:, h0:h0 + BH],
                                            e_ps[:, 1, :], op=ALU.add)
                elif hc == 2:
                    nc.vector.tensor_tensor(et_sb[:, h0:h0 + BH],
                                            et_sb[:, h0:h0 + BH],
                                            e_ps[:, 2, :], op=ALU.add)
                elif hc == 3:
                    nc.vector.tensor_tensor(et_sb[r0:r1, h0:h0 + BH],
                                            et_sb[r0:r1, h0:h0 + BH],
                                            e_ps[r0:r1, 3, :], op=ALU.add)

            def emit_gen_mm(k0, nk, off, width):
                m = B * nk
                s0 = k0 % 16
                o_ps = psO.tile([128, NSUB], F32, tag="gen")
                for hc in range(HC):
                    nc.tensor.matmul(
                        o_ps[:m, 0:width],
                        hist[:, hc, s0:s0 + nk, :],
                        wgen_sb[:, hc, off:off + width],
                        start=(hc == 0), stop=False)
                nc.tensor.matmul(
                    o_ps[:m, 0:width], ones1b_sb[0:1, 0:m],
                    bgen_sb[0:1, off:off + width], start=False, stop=True)
                return o_ps, m

            def emit_gen_out(o_ps, m, k0, nk, off, width, on_act=True,
                             ring=None):
                o_sb = gop.tile([128, NSUB], F32, tag="ob")
                if on_act:
                    nc.scalar.activation(o_sb[:m, 0:width], o_ps[:m, 0:width],
                                         AF.Copy)
                else:
                    nc.vector.tensor_copy(o_sb[:m, 0:width], o_ps[:m, 0:width])
                if ring is None:
                    genct[0] += 1
                    ring = nc.sync if genct[0] % 2 == 0 else nc.scalar
                ring.dma_start(
                    out[k0:k0 + nk, :, off:off + width], o_sb[:m, 0:width])

            # ---- tail sub-blocks for half at h0 (its z-phase ran last
            #      phase; e_ps columns 0-2 are already summed in et_sb) ----
            def tail_softmax(h0):
                st = state[h0 // BH]
                nc.scalar.activation(E_sb[:, h0:h0 + BH], et_sb[:, h0:h0 + BH],
                                     AF.Exp)
                s_ps = psS.tile([1, BH], F32, tag="sm")
                nc.tensor.matmul(s_ps[:, :], ones128_sb[:, :],
                                 E_sb[:, h0:h0 + BH], start=True, stop=True)
                with nc.allow_low_precision(reason="1/s bf16; ctx is bf16"):
                    nc.vector.reciprocal(recip_sb[:, h0:h0 + BH], s_ps[:, :])
                bc_ps = psS.tile([128, 1, BH], F32, tag="sm")
                nc.tensor.matmul(bc_ps[:, 0, :], ones1b_sb[0:1, :],
                                 recip_sb[:, h0:h0 + BH], start=True, stop=True)
                nc.vector.tensor_copy(bc_sb[:, :, h0:h0 + BH], bc_ps[:, :, :])

            def tail_ctx_mm(h0):
                st = state[h0 // BH]
                if "ctx" in skip:
                    return
                ctx_ps = psS.tile([128, CC, BH], F32, tag="sm")
                st["ctx"] = ctx_ps
                for b in range(BH):
                    for cc in range(CC):
                        nc.tensor.matmul(
                            ctx_ps[:, cc, b:b + 1],
                            fs_sb[:, h0 + b, cc * 128:(cc + 1) * 128],
                            E_sb[:, h0 + b:h0 + b + 1],
                            start=True, stop=True)

            def tail_ctx_evac(h0):
                if "ctx" in skip:
                    nc.vector.memset(ctxT_sb[:, :, h0:h0 + BH], 0.01)
                    return
                ctx_ps = state[h0 // BH]["ctx"]
                # evac with the 1/s scale folded in
                nc.vector.tensor_tensor(
                    ctxT_sb[:, :, h0:h0 + BH], ctx_ps[:, :, :],
                    bc_sb[:, :, h0:h0 + BH].broadcast_to((128, CC, BH)),
                    op=ALU.mult)

            def tail_gates_ih(h0):
                if "gates" in skip:
                    return
                g_ps = state[h0 // BH]["g"]
                for mi in range(12, 24):
                    col = ((mi - 12) % 4) * 128 + (0 if mi < 16 else
                                                   512 if mi < 20 else 1024)
                    for kc in range(CC):
                        nc.tensor.matmul(
                            g_ps[:, mi, :], wih_sb[:, kc, col:col + 128],
                            ctxT_sb[:, kc, h0:h0 + BH],
                            start=(kc == 0), stop=False)
                    boff = (mi - 12) * 128 if mi < 20 else 1024 + (mi - 20) * 128
                    nc.tensor.matmul(
                        g_ps[:, mi, :], brow_sb[0:1, boff:boff + 128],
                        ones1b_sb[0:1, 0:BH], start=False, stop=True)
                # rz preactivation = hh part + ih part (the hh evac runs
                # early, off-chain: its matmuls finished at phase start)
                nc.vector.tensor_copy(g2_sb[:, :, h0:h0 + BH], g_ps[:, 0:8, :])
                nc.vector.tensor_tensor(g2_sb[:, :, h0:h0 + BH],
                                        g2_sb[:, :, h0:h0 + BH],
                                        g_ps[:, 12:20, :], op=ALU.add)
                nc.scalar.activation(rz_sb[:, :, h0:h0 + BH],
                                     g2_sb[:, :, h0:h0 + BH], AF.Tanh,
                                     scale=0.5)

            def tail_n_h(h0, k):
                if "gates" in skip:
                    nc.vector.memset(hist[:, :, k % 16, h0:h0 + BH], 0.01)
                    return
                g_ps = state[h0 // BH]["g"]
                hprev = hist[:, :, (k - 1) % 16, h0:h0 + BH]
                t1 = t1_sb[:, :, h0:h0 + BH]
                n = n_sb[:, :, h0:h0 + BH]
                # n = tanh(0.5*(A' + (t_r+1)*gh_n)), A' = 2*(W_ih,n ctx + b)
                nc.vector.scalar_tensor_tensor(
                    t1, rz_sb[:, 0:4, h0:h0 + BH], 1.0, g_ps[:, 8:12, :],
                    ALU.add, ALU.mult)
                nc.vector.tensor_tensor(t1, t1, g_ps[:, 20:24, :], op=ALU.add)
                nc.scalar.activation(n, t1, AF.Tanh, scale=0.5)
                # h' = n + 0.5*(t_z+1)*(h-n)
                nc.vector.tensor_tensor(t1, hprev, n, op=ALU.subtract)
                nc.vector.scalar_tensor_tensor(
                    t1, rz_sb[:, 4:8, h0:h0 + BH], 1.0, t1, ALU.add, ALU.mult)
                nc.vector.scalar_tensor_tensor(
                    hist[:, :, k % 16, h0:h0 + BH], t1, 0.5, n,
                    ALU.mult, ALU.add)

            def emit_phase(h0, k, tail_h0, tail_k, gen_items):
                """z-phase for half h0 at step k, with the other half's tail
                (for step tail_k) interleaved at fixed points."""
                me = state[h0 // BH] = {}
                do_tail = tail_k >= 0 and "tail" not in skip
                emit_hp(h0, k)
                emit_gates_hh(h0, k)
                e_ps = me["g"][:, 24:28, :]
                me["e"] = e_ps
                if do_tail:
                    # softmax + ctx matmuls first: the exp->1/s->ctx->gates
                    # chain is ~4.5us and must finish before rz (after z2)
                    tail_softmax(tail_h0)
                    tail_ctx_mm(tail_h0)
                zts = [emit_z(h0, 0, split=True)]
                zts.append(emit_z(h0, 1))
                if do_tail:
                    tail_ctx_evac(tail_h0)
                zts.append(emit_z(h0, 2))
                if do_tail:
                    tail_gates_ih(tail_h0)
                emit_e(zts[0], 0, e_ps, h0)
                emit_e(zts[1], 1, e_ps, h0)
                if do_tail:
                    tail_n_h(tail_h0, tail_k)
                zts.append(emit_z(h0, 3, split=True))
                emit_e(zts[2], 2, e_ps, h0)
                gen_work = [emit_gen_mm(*gi) + gi for gi in gen_items]
                emit_e(zts[3], 3, e_ps, h0, half=0)
                emit_e(zts[3], 3, e_ps, h0, half=1)
                # gen evacuations land in the boundary gap while DVE runs
                # the next phase's first z-add
                for (o_ps, m, k0, nk, off, width) in gen_work:
                    emit_gen_out(o_ps, m, k0, nk, off, width)

            def emit_tail(h0, k):
                tail_softmax(h0)
                tail_ctx_mm(h0)
                tail_ctx_evac(h0)
                tail_gates_ih(h0)
                tail_n_h(h0, k)

            from contextlib import ExitStack
            for rep in range(reps):
                if rep > 0:
                    nc.vector.memset(hist[:, :, 15, :], 0.0)
                state[0] = {}
                state[1] = {}
                with ExitStack() as scan_pools:
                    zp = scan_pools.enter_context(
                        tc.tile_pool(name=f"zp{rep}", bufs=6))
                    gop = scan_pools.enter_context(
                        tc.tile_pool(name=f"gop{rep}", bufs=3))
                    psS = scan_pools.enter_context(
                        tc.tile_pool(name=f"psS{rep}", bufs=3, space="PSUM"))
                    psG = scan_pools.enter_context(
                        tc.tile_pool(name=f"psG{rep}", bufs=2, space="PSUM"))
                    psO = scan_pools.enter_context(
                        tc.tile_pool(name=f"psO{rep}", bufs=3, space="PSUM"))
                    for k in range(L):
                        gi = [] if "gen" in skip else GEN_SCHED[k]
                        # a group's first emission step: the B half's h for
                        # the group's last step lands mid-A-phase; emitting
                        # its chunks in the A phase would head-of-line block
                        # PE behind that write, so they all go to the B phase
                        if any(k0 + nk == k for (k0, nk, _, _) in gi):
                            ga, gb = [], gi
                        else:
                            ga, gb = gi[:len(gi) // 2], gi[len(gi) // 2:]
                        emit_phase(0, k, BH, k - 1, ga)
                        emit_phase(BH, k, 0, k, gb)
                    if "tail" not in skip:
                        emit_tail(BH, L - 1)

                # ---- generator tail: rows for steps 24..25, on wide pools
                #      so the 14 chunks pipeline instead of serializing ----
                with (
                    tc.tile_pool(name=f"gopD{rep}", bufs=8) as gopD,
                    tc.tile_pool(name=f"psD{rep}", bufs=8, space="PSUM") as psD,
                ):
                    gop, psO = gopD, psD
                    for i, (k0, nk, off, width) in enumerate(
                            [] if "gen" in skip else GEN_TAIL):
                        o_ps, m = emit_gen_mm(k0, nk, off, width)
                        emit_gen_out(o_ps, m, k0, nk, off, width,
                                     on_act=(i % 2 == 0),
                                     ring=(nc.sync if i % 2 == 0 else
                                           nc.scalar))

    nc.finalize()
    return nc


def _get_graph():
    if not hasattr(_get_graph, "_nc"):
        _get_graph._nc = build_graph()
    return _get_graph._nc


def make_in_maps(feats, text_length, W_i2h, W_h2h, b_h2h, W_score, W_ih, W_hh,
                 b_ih, b_hh, W_gen, b_gen):
    bf = ml_dtypes.bfloat16
    feats = np.asarray(feats, np.float32)

    wi2h = np.ascontiguousarray(np.asarray(W_i2h, np.float32).T).astype(bf)
    wh2h = np.ascontiguousarray(np.asarray(W_h2h, np.float32).T).astype(bf)
    bh2hr = np.asarray(b_h2h, np.float32)[None, :].astype(bf)
    wcol = np.ascontiguousarray(
        np.asarray(W_score, np.float32)[0].reshape(HC, 128).T).astype(bf)
    # n-gate input weights/bias doubled: n = tanh(0.5*(A' + (t_r+1)*gh_n))
    wih_f = np.ascontiguousarray(np.asarray(W_ih, np.float32).T).copy()
    wih_f[:, 2 * H:] *= 2.0
    wih = wih_f.astype(bf)
    whh = np.ascontiguousarray(np.asarray(W_hh, np.float32).T).astype(bf)
    b_ih = np.asarray(b_ih, np.float32)
    b_hh = np.asarray(b_hh, np.float32)
    brow1 = np.concatenate([b_ih[:2 * H] + b_hh[:2 * H],
                            2.0 * b_ih[2 * H:], b_hh[2 * H:]])
    brow = brow1[None, :].astype(bf)
    wgen = np.ascontiguousarray(np.asarray(W_gen, np.float32).T).astype(bf)
    bgen = np.asarray(b_gen, np.float32)[None, :].astype(bf)

    in_maps = []
    for c in range(NCORES):
        fsh = feats[:, c * B:(c + 1) * B, :]                     # [T, B, C]
        featsT = np.ascontiguousarray(fsh.transpose(2, 1, 0)).astype(bf)
        featsS = np.ascontiguousarray(fsh.transpose(1, 0, 2)).astype(bf)
        in_maps.append({
            "featsT": featsT, "featsS": featsS, "wi2h": wi2h, "wh2h": wh2h,
            "bh2hr": bh2hr, "wcol": wcol, "wih": wih, "whh": whh,
            "brow": brow, "wgen": wgen, "bgen": bgen,
        })

    return in_maps


def kernel(**inputs):
    nc = _get_graph()
    in_maps = make_in_maps(**inputs)
    res = run_bass_kernel_spmd(nc, in_maps, core_ids=list(range(NCORES)))
    return np.concatenate(
        [res.results[c]["out"].transpose(1, 0, 2).reshape(B * L, NCLS)
         for c in range(NCORES)],
        axis=0)
